# revision 41
# baseline (speedup 1.0000x reference)
"""Trainium2 Bass kernel for nn_AttRouter (MoE attention routing).

Reference computation (per token t, E=16 experts, D=256, A=64):
    Q = Xq @ Wq                  # (E, A)
    K = Xk @ Wk                  # (E, A)
    sim = softmax_k(Q K^T / 8)   # (E, E)
    gate = sim @ g               # (E,)
    out = softmax_E(gate)

Strategy (8 cores, data-parallel over tokens; 1024 tokens = 16384
token-expert rows per core):
  - Load X in natural layout [rows, D], PE-transpose 128x128 blocks,
    project with D on partitions: QT/KT = W^T @ X^T -> [A, rows].
  - Per 128-row tile (8 tokens): simT[(t,k),(t,q)] = KT^T QT via PE with a
    block-diagonal -BIG mask added by a rank-8 matmul of constant one-hot
    factors (kills cross-token terms after exp).
  - exp on ACT (scale 1/8); numerator+denominator in one PE matmul
    against interleaved [g | 1] columns; all 128 tiles' [num|den] pairs
    accumulate in one PSUM bank.
  - Final routing softmax over the 16 experts (partition-block dim) done
    once per core with a handful of PE/DVE/ACT ops on [128, ntiles] data.

Dispatch-overhead design (the dominant cost under the axon tunnel —
device main-loop time is ~64us/core and fully hidden by pipelining;
per-call PJRT/relay overhead is ~0.65ms and sets the floor):
  - ALL per-core inputs (xq|xk interleaved, gate, weights, constants) are
    packed host-side into ONE dram tensor [16608, 512] -> a call carries
    2 operands (x, out) instead of 8 (~60us/operand/call saved).
  - bf16 mask constants are derived on device from the f32 consts.
  - Output operand buffers are created on device once and reused every
    call (kernel overwrites every element; no per-call h2d transfer).
  - partition_id disabled (unused); callable AOT-compiled at init.

Self-contained: hardcodes shapes/sharding; no file reads.
"""

import numpy as np

B, S, E, D, A = 4, 2048, 16, 256, 64
# One 8-core mesh per call. Measured tradeoff: per-call dispatch is
# ~470-510us for 8 devices vs ~370us for 2, but the axon relay keeps only
# ~2 calls in flight, so fewer-core variants (device time 1.5ms/pair)
# cannot amortize their device time across pipelined calls — 8-way keeps
# per-core device time (~350us) hidden under the dispatch floor.
N_CORES = 8                      # cores per call (one mesh)
N_MESHES = 1                     # disjoint meshes among the 8 cores
TOK = B * S                      # 8192 tokens
ROWS = TOK * E                   # 131072 token-expert rows
ROWS_CORE = ROWS // N_CORES      # 16384 rows per core
NTILES = ROWS_CORE // 128        # 128 tiles of 128 rows
BIG = 240.0                      # additive mask; -240/8 = -30 pre-exp

# ---- perf/precision knobs ----
CHUNK_TILES = 8                  # tiles per DMA chunk (8 -> 2 MiB/chunk)
GROUP = 4                        # tiles per projection group (N=512 matmuls)
PROJ_F32R = True                 # float32r projections (full-rate PE)
TRANS_F32R = False               # float32r transposes via SWDGE cast loads
TRANS_BITCAST = True             # bitcast f32 inputs to f32r for the PE
                                 # transposes (full rate, no SWDGE)
SIM_DT = "f32r"                  # QT/KT + sim matmul dtype: f32|f32r|bf16
                                 # (f32r: full-rate PE, device loop 378
                                 # -> 316us; trans=f32r via SWDGE is a
                                 # net LOSS on hw, keep TRANS_F32R off)
EXP_DT = "f32"                   # exp output (numden lhsT): f32|f32r|bf16
NO_COMPUTE = False               # DMA-only variant (bandwidth floor probe)
NO_TRANS = False                 # skip PE transposes (diagnostic, wrong data)
PS_T_BUFS = 2                    # transpose psum pool bufs
PS_SIM_BUFS = 2                  # sim psum pool bufs (PSUM budget: ps_t 2
                                 # + ps_qk 2 + ps_sim 2 + ps_nd 2 = 8 banks)
REPEAT = 1                       # run the main loop R times (bench slope)


def _make_consts():
    """f32 consts [128, 392]: identity | maskA | maskB | onehot8."""
    c = np.zeros((128, 392), np.float32)
    c[:, 0:128] = np.eye(128, dtype=np.float32)
    r = np.arange(128)
    # maskA[j, row] = 1 where row's token (row//16) == j  (rows 0..7 used)
    c[r // 16, 128 + r] = 1.0
    # maskB[j, row] = -BIG where row//16 != j (rows 0..7 used)
    mb = np.full((8, 128), -BIG, np.float32)
    mb[r // 16, r] = 0.0
    c[0:8, 256:384] = mb
    # onehot8[row, row//16] = 1
    c[r, 384 + r // 16] = 1.0
    return c


def _pack_rows(ntiles):
    """Row offsets of the packed single input tensor [rows, 512] f32:
    xq|xk (nrows x [256|256]), then g, wq, wk, consts as flat rows."""
    nrows = ntiles * 128
    g_rows = ntiles * 128 // 512
    r_g = nrows
    r_wq = r_g + g_rows
    r_wk = r_wq + 32
    r_c = r_wk + 32
    r_end = r_c + 128
    return r_g, r_wq, r_wk, r_c, r_end


def build_program(ntiles=NTILES):
    import concourse.bacc as bacc
    import concourse.tile as tile
    from concourse import mybir

    f32 = mybir.dt.float32
    f32r = mybir.dt.float32r
    bf16 = mybir.dt.bfloat16
    dts = {"f32": f32, "f32r": f32r, "bf16": bf16}
    simdt = dts[SIM_DT]
    expdt = dts[EXP_DT]
    Exp = mybir.ActivationFunctionType.Exp

    nrows = ntiles * 128
    nchunks = ntiles // CHUNK_TILES
    groups_per_chunk = CHUNK_TILES // GROUP

    nc = bacc.Bacc("TRN2", enable_partition_id=False)
    r_g, r_wq, r_wk, r_c, r_end = _pack_rows(ntiles)
    x_d = nc.dram_tensor("x", [r_end, 512], f32, kind="ExternalInput")
    g_d = x_d[r_g:r_wq, :].rearrange("r (a c) -> (r a) c", c=128)
    wq_d = x_d[r_wq:r_wk, :].rearrange("r (a c) -> (r a) c", c=128)
    wk_d = x_d[r_wk:r_c, :].rearrange("r (a c) -> (r a) c", c=128)
    consts_d = x_d[r_c:r_end, 0:392]
    out_d = nc.dram_tensor("out", [ntiles, 128], f32, kind="ExternalOutput")

    def r32(ap):
        return ap.bitcast(f32r)

    with tile.TileContext(nc) as tc:
        with (
            tc.tile_pool(name="singles", bufs=1) as singles,
            tc.tile_pool(name="p_in", bufs=2) as p_in,
            tc.tile_pool(name="p_xt", bufs=2) as p_xt,
            tc.tile_pool(name="p_qk", bufs=2) as p_qk,
            tc.tile_pool(name="p_exp", bufs=3) as p_exp,
            tc.tile_pool(name="p_fin", bufs=1) as p_fin,
            tc.tile_pool(name="ps_t", bufs=PS_T_BUFS, space="PSUM") as ps_t,
            tc.tile_pool(name="ps_qk", bufs=1, space="PSUM") as ps_qk,
            tc.tile_pool(name="ps_sim", bufs=PS_SIM_BUFS, space="PSUM")
            as ps_sim,
            tc.tile_pool(name="ps_nd", bufs=1, space="PSUM") as ps_nd,
        ):
            # ---- one-time setup ----
            consts = singles.tile([128, 392], f32)
            nc.sync.dma_start(consts, consts_d)
            ident = consts[:, 0:128]
            # masks always bf16 (0 / -240 are exact in bf16; 4x faster MM);
            # built on device from the f32 consts to save an input operand
            consts16 = singles.tile([8, 256], bf16)
            nc.vector.tensor_copy(consts16[:, 0:128], consts[0:8, 256:384])
            nc.vector.tensor_copy(consts16[:, 128:256], consts[0:8, 128:256])
            maskB, maskA = consts16[:, 0:128], consts16[:, 128:256]
            oh8 = consts[:, 384:392]
            oh8T = consts[0:8, 128:256]          # == maskA rows: delta(t=j)

            wq_sb = singles.tile([128, 128], f32)
            wk_sb = singles.tile([128, 128], f32)
            nc.sync.dma_start(wq_sb, wq_d)
            nc.sync.dma_start(wk_sb, wk_d)
            if PROJ_F32R:
                wq_r = singles.tile([128, 128], f32r)
                wk_r = singles.tile([128, 128], f32r)
                nc.vector.tensor_copy(wq_r, wq_sb)
                nc.vector.tensor_copy(wk_r, wk_sb)
                wq_sb, wk_sb = wq_r, wk_r
            if TRANS_F32R or TRANS_BITCAST:
                ident_t = singles.tile([128, 128], f32r)
                nc.vector.tensor_copy(ident_t, ident)
            else:
                ident_t = ident

            # gate: load [ntiles, 128] in <=128-row blocks, PE-transpose
            # each into gT [128, ntiles] (PSUM), then interleave with ones
            gT_ps = ps_sim.tile([128, ntiles], f32, tag="sim")
            for blk in range(0, ntiles, 128):
                nb = min(128, ntiles - blk)
                g_sb = singles.tile([nb, 128], f32, tag=f"g{blk}")
                nc.sync.dma_start(g_sb, g_d[blk:blk + nb, :])
                nc.tensor.transpose(gT_ps[:, blk:blk + nb], g_sb,
                                    ident[0:nb, 0:nb])
            gones = singles.tile([128, 2 * ntiles], expdt)
            gones_v = gones.rearrange("p (i two) -> p i two", two=2)
            nc.vector.tensor_copy(gones_v[:, :, 0], gT_ps)
            nc.vector.memset(gones_v[:, :, 1], 1.0)

            num_den_ps = ps_nd.tile([128, 2 * ntiles], f32)

            # ---- main loop ----
            indt = f32r if TRANS_F32R else f32
            tdt = f32r if (TRANS_F32R or TRANS_BITCAST) else f32
            dma_in = nc.gpsimd if TRANS_F32R else nc.sync
            for c in [ci for _ in range(REPEAT) for ci in range(nchunks)]:
                r0 = c * CHUNK_TILES * 128
                r1 = r0 + CHUNK_TILES * 128
                # one DMA for q|k together: 2KB contiguous per (p, n) line
                inx = p_in.tile([128, CHUNK_TILES * 512], indt, tag="inx")
                dma_in.dma_start(
                    inx.rearrange("p (n d) -> p n d", d=512),
                    x_d[r0:r1, :].rearrange("(n p) d -> p n d", p=128))

                if NO_COMPUTE:
                    sink = p_exp.tile([1, 8], f32, tag="sink")
                    nc.vector.tensor_copy(sink[:, 0:8], inx[0:1, 0:8])
                    continue

                if TRANS_BITCAST and not TRANS_F32R:
                    # round the chunk to f32r on the (mostly idle) ACT
                    # engine so the PE transposes run at full rate
                    inxr = p_in.tile([128, CHUNK_TILES * 512], f32r,
                                     tag="inxr")
                    nc.scalar.activation(inxr, inx,
                                         mybir.ActivationFunctionType.Copy,
                                         scale=1.0)
                    inx = inxr

                for gi in range(groups_per_chunk):
                    xtdt = f32r if PROJ_F32R else f32
                    xqT = p_xt.tile([128, GROUP * 256], xtdt, tag="xqT")
                    xkT = p_xt.tile([128, GROUP * 256], xtdt, tag="xkT")
                    for j in range(GROUP):
                        jj = gi * GROUP + j
                        if NO_TRANS:
                            # diagnostic: same copies, wrong (untransposed)
                            # data, no PE transposes
                            nc.vector.tensor_copy(
                                xqT[:, j * 256:(j + 1) * 256],
                                inx[:, jj * 512:jj * 512 + 256])
                            nc.vector.tensor_copy(
                                xkT[:, j * 256:(j + 1) * 256],
                                inx[:, jj * 512 + 256:(jj + 1) * 512])
                            continue
                        pT = ps_t.tile([128, 512], tdt, tag="pT")
                        for h in range(2):
                            src_q = inx[:, jj * 512 + h * 128:
                                        jj * 512 + (h + 1) * 128]
                            src_k = inx[:, jj * 512 + 256 + h * 128:
                                        jj * 512 + 256 + (h + 1) * 128]
                            nc.tensor.transpose(
                                pT[:, h * 128:(h + 1) * 128], src_q, ident_t)
                            nc.tensor.transpose(
                                pT[:, 256 + h * 128:256 + (h + 1) * 128],
                                src_k, ident_t)
                        nc.vector.tensor_copy(
                            xqT[:, j * 256:(j + 1) * 256], pT[:, 0:256])
                        nc.vector.tensor_copy(
                            xkT[:, j * 256:(j + 1) * 256], pT[:, 256:512])

                    # projections: QT/KT [64, GROUP*128], separate banks
                    qt_ps = ps_qk.tile([64, GROUP * 128], f32, tag="qt")
                    kt_ps = ps_qk.tile([64, GROUP * 128], f32, tag="kt")
                    xqT_v = xqT.rearrange("p (j h d) -> p h j d", h=2, d=128)
                    xkT_v = xkT.rearrange("p (j h d) -> p h j d", h=2, d=128)
                    for h in range(2):
                        nc.tensor.matmul(qt_ps,
                                         wq_sb[:, h * 64:(h + 1) * 64],
                                         xqT_v[:, h],
                                         start=(h == 0), stop=(h == 1))
                    for h in range(2):
                        nc.tensor.matmul(kt_ps,
                                         wk_sb[:, h * 64:(h + 1) * 64],
                                         xkT_v[:, h],
                                         start=(h == 0), stop=(h == 1))
                    qt_sb = p_qk.tile([64, GROUP * 128], simdt, tag="qtsb")
                    kt_sb = p_qk.tile([64, GROUP * 128], simdt, tag="ktsb")
                    nc.vector.tensor_copy(qt_sb, qt_ps)
                    nc.vector.tensor_copy(kt_sb, kt_ps)

                    for j in range(GROUP):
                        i = c * CHUNK_TILES + gi * GROUP + j
                        sl = slice(j * 128, (j + 1) * 128)
                        sim_ps = ps_sim.tile([128, 128], f32, tag="sim")
                        nc.tensor.matmul(sim_ps, maskB, maskA,
                                         start=True, stop=False)
                        nc.tensor.matmul(sim_ps, kt_sb[:, sl], qt_sb[:, sl],
                                         start=False, stop=True)
                        exp_t = p_exp.tile([128, 128], expdt, tag="exp")
                        nc.scalar.activation(exp_t, sim_ps, Exp, scale=0.125)
                        nc.tensor.matmul(num_den_ps[:, 2 * i:2 * i + 2],
                                         exp_t, gones[:, 2 * i:2 * i + 2],
                                         start=True, stop=True)

            # ---- final routing softmax over experts ----
            nd_sb = p_fin.tile([128, 2 * ntiles], f32)
            if NO_COMPUTE:
                nc.vector.memset(nd_sb, 1.0)
            else:
                nc.vector.tensor_copy(nd_sb, num_den_ps)
            nd_v = nd_sb.rearrange("p (i two) -> p i two", two=2)
            recd = p_fin.tile([128, ntiles], f32)
            nc.vector.reciprocal(recd, nd_v[:, :, 1])
            eg = p_fin.tile([128, ntiles], f32)
            # egate = exp(num * (1/den)); gate in (0,1) so no max-subtract
            gate = p_fin.tile([128, ntiles], f32)
            nc.vector.tensor_mul(gate, nd_v[:, :, 0], recd)
            nc.scalar.activation(eg, gate, Exp, scale=1.0)

            # per-128-column blocks: expert-sums, reciprocal, transpose
            rsT_sb = p_fin.tile([8, ntiles], f32)
            for blk in range(0, ntiles, 128):
                nb = min(128, ntiles - blk)
                sums_ps = ps_sim.tile([nb, 8], f32, tag="sim")
                nc.tensor.matmul(sums_ps, eg[:, blk:blk + nb], oh8,
                                 start=True, stop=True)
                rs = p_fin.tile([nb, 8], f32, tag=f"rs{blk}")
                nc.vector.reciprocal(rs, sums_ps)
                rsT_ps = ps_sim.tile([8, nb], f32, tag="sim")
                nc.tensor.transpose(rsT_ps, rs, ident[0:nb, 0:nb])
                nc.vector.tensor_copy(rsT_sb[:, blk:blk + nb], rsT_ps)
            bc_ps = ps_sim.tile([128, ntiles], f32, tag="sim")
            nc.tensor.matmul(bc_ps, oh8T, rsT_sb, start=True, stop=True)
            rout = p_fin.tile([128, ntiles], f32)
            nc.vector.tensor_mul(rout, eg, bc_ps)
            for blk in range(0, ntiles, 128):
                nb = min(128, ntiles - blk)
                routT_ps = ps_sim.tile([nb, 128], f32, tag="sim")
                nc.tensor.transpose(routT_ps, rout[:, blk:blk + nb], ident)
                routT_sb = p_fin.tile([nb, 128], f32, tag=f"rt{blk}")
                nc.vector.tensor_copy(routT_sb, routT_ps)
                nc.sync.dma_start(out_d[blk:blk + nb, :], routT_sb)
    return nc


def _pack_full(input_gate, q_expert_vector, k_expert_vector, Wq, Wk,
               ntiles=NTILES):
    """One contiguous packed input [N_CORES * r_end, 512] (single copy)."""
    xq = np.asarray(q_expert_vector, np.float32).reshape(ROWS, D)
    xk = np.asarray(k_expert_vector, np.float32).reshape(ROWS, D)
    g = np.asarray(input_gate, np.float32).reshape(ROWS)
    wq = np.asarray(Wq, np.float32).reshape(2, 128, A).transpose(1, 0, 2) \
        .reshape(128, 128)
    wk = np.asarray(Wk, np.float32).reshape(2, 128, A).transpose(1, 0, 2) \
        .reshape(128, 128)
    consts_pad = np.zeros((128, 512), np.float32)
    consts_pad[:, 0:392] = _make_consts()
    nrows = ntiles * 128
    r_g, r_wq, r_wk, r_c, r_end = _pack_rows(ntiles)
    xf = np.empty((N_CORES, r_end, 512), np.float32)
    if ntiles == NTILES:
        xf[:, 0:nrows, 0:256] = xq.reshape(N_CORES, nrows, D)
        xf[:, 0:nrows, 256:512] = xk.reshape(N_CORES, nrows, D)
        xf[:, r_g:r_wq] = g.reshape(N_CORES, -1, 512)
    else:
        for c in range(N_CORES):
            r0 = c * ROWS_CORE
            xf[c, 0:nrows, 0:256] = xq[r0:r0 + nrows]
            xf[c, 0:nrows, 256:512] = xk[r0:r0 + nrows]
            xf[c, r_g:r_wq] = g[r0:r0 + nrows].reshape(-1, 512)
    xf[:, r_wq:r_wk] = wq.reshape(32, 512)
    xf[:, r_wk:r_c] = wk.reshape(32, 512)
    xf[:, r_c:r_end] = consts_pad
    return xf.reshape(N_CORES * r_end, 512)


def _shard_inputs(input_gate, q_expert_vector, k_expert_vector, Wq, Wk,
                  ntiles=NTILES):
    xf = _pack_full(input_gate, q_expert_vector, k_expert_vector, Wq, Wk,
                    ntiles)
    r_end = _pack_rows(ntiles)[-1]
    xf = xf.reshape(N_CORES, r_end, 512)
    return [{"x": xf[c]} for c in range(N_CORES)]


_CACHE = {}


def _get_program(ntiles=NTILES):
    key = (ntiles, CHUNK_TILES, GROUP, PROJ_F32R, TRANS_F32R, TRANS_BITCAST,
           SIM_DT, EXP_DT, NO_COMPUTE, REPEAT)
    if key not in _CACHE:
        nc = build_program(ntiles)
        nc.finalize()
        _CACHE[key] = nc
    return _CACHE[key]


class _Runner:
    """Cached SPMD PJRT runner (compiles once; repeat calls are cheap).

    Mirrors concourse.bass2jax.run_bass_via_pjrt but keeps the jitted
    callable and mesh alive across calls.
    """

    def __init__(self, nc):
        import jax
        from jax.experimental.shard_map import shard_map
        from jax.sharding import Mesh, PartitionSpec
        from concourse import bass2jax, mybir
        bass2jax.install_neuronx_cc_hook()
        self.jax = jax
        partition_name = (nc.partition_id_tensor.name
                          if nc.partition_id_tensor else None)
        in_names, out_names, out_avals, zero_outs = [], [], [], []
        in_avals = []
        for alloc in nc.m.functions[0].allocations:
            if not isinstance(alloc, mybir.MemoryLocationSet):
                continue
            name = alloc.memorylocations[0].name
            if alloc.kind == "ExternalInput":
                if name != partition_name:
                    in_names.append(name)
                    in_avals.append(jax.core.ShapedArray(
                        tuple(alloc.tensor_shape), mybir.dt.np(alloc.dtype)))
            elif alloc.kind == "ExternalOutput":
                out_names.append(name)
                shape = tuple(alloc.tensor_shape)
                dtype = mybir.dt.np(alloc.dtype)
                out_avals.append(jax.core.ShapedArray(shape, dtype))
                zero_outs.append(np.zeros(shape, dtype))
        self.in_names, self.out_names = list(in_names), list(out_names)
        self.in_avals = in_avals
        self.out_avals, self.zero_outs = out_avals, zero_outs
        n_params, n_outs = len(in_names), len(out_names)
        all_names = list(in_names) + list(out_names)
        if partition_name is not None:
            all_names.append(partition_name)
        all_names = tuple(all_names)

        self._bass2jax = bass2jax
        self._nc = nc
        self._all_names = all_names
        self._partition_name = partition_name
        self._n_params, self._n_outs = n_params, n_outs
        # One mesh per disjoint core pair. Mesh 0 (used by kernel()) is
        # compiled eagerly; the rest lazily on first use (timing loop).
        self._meshes = [None] * N_MESHES
        self._get_mesh(0)

    def _get_mesh(self, m):
        """(compiled_callable, sharding, dev_zeros) for pair m."""
        if self._meshes[m] is not None:
            return self._meshes[m]
        import jax
        from jax.experimental.shard_map import shard_map
        from jax.sharding import Mesh, NamedSharding, PartitionSpec
        bass2jax, nc = self._bass2jax, self._nc
        out_avals, out_names = self.out_avals, self.out_names
        all_names, partition_name = self._all_names, self._partition_name

        def _body(*args):
            operands = list(args)
            if partition_name is not None:
                operands.append(bass2jax.partition_id_tensor())
            outs = bass2jax._bass_exec_p.bind(
                *operands,
                out_avals=tuple(out_avals),
                in_names=all_names,
                out_names=tuple(out_names),
                lowering_input_output_aliases=(),
                sim_require_finite=True,
                sim_require_nnan=True,
                nc=nc,
            )
            return tuple(outs)

        devices = jax.devices()[m * N_CORES:(m + 1) * N_CORES]
        mesh = Mesh(np.asarray(devices), ("core",))
        specs = (PartitionSpec("core"),) * (self._n_params + self._n_outs)
        sharded = jax.jit(
            shard_map(_body, mesh=mesh, in_specs=specs,
                      out_specs=(PartitionSpec("core"),) * self._n_outs,
                      check_rep=False),
            keep_unused=True,
        )
        sh = NamedSharding(mesh, PartitionSpec("core"))
        # Output operand buffers, created on device ONCE and reused every
        # call (no donation). The kernel overwrites every output element,
        # so stale contents are irrelevant; this avoids a per-call
        # host->device zeros transfer over the axon tunnel (~16ms/iter).
        dev_zeros = [
            jax.device_put(
                np.zeros((N_CORES * z.shape[0], *z.shape[1:]), z.dtype), sh)
            for z in self.zero_outs]
        # AOT-compile with bass_effect suppressed: enables jax's C++ pjit
        # fast-path dispatch (the effectful path forces a ~250us/call
        # python fallback + token handling).
        in_sds = [jax.ShapeDtypeStruct(
            (N_CORES * a.shape[0], *a.shape[1:]), a.dtype, sharding=sh)
            for a in self.in_avals]
        out_sds = [jax.ShapeDtypeStruct(
            (N_CORES * z.shape[0], *z.shape[1:]), z.dtype, sharding=sh)
            for z in self.zero_outs]
        compiled = bass2jax.fast_dispatch_compile(
            lambda: sharded.lower(*in_sds, *out_sds).compile())
        self._meshes[m] = (compiled, sh, dev_zeros)
        return self._meshes[m]

    def put_inputs(self, in_maps, mesh_idx=0):
        """Concat per-core inputs and move them to a pair mesh."""
        import jax
        _, sh, _ = self._get_mesh(mesh_idx)
        out = []
        for name in self.in_names:
            arr = np.concatenate([np.asarray(m[name]) for m in in_maps],
                                 axis=0)
            out.append(jax.device_put(arr, sh))
        return out

    def put_full(self, full_arrays, mesh_idx=0):
        """Move pre-packed full input arrays (ordered as in_names) to a
        pair mesh without any host-side copy."""
        import jax
        _, sh, _ = self._get_mesh(mesh_idx)
        return [jax.device_put(a, sh) for a in full_arrays]

    def run_device(self, dev_inputs, mesh_idx=0):
        compiled, _, dev_zeros = self._get_mesh(mesh_idx)
        return compiled(*dev_inputs, *dev_zeros)

    def __call__(self, in_maps):
        outs = self.run_device(self.put_inputs(in_maps))
        res = []
        for c in range(N_CORES):
            res.append({
                name: np.asarray(outs[i]).reshape(
                    N_CORES, *self.out_avals[i].shape)[c]
                for i, name in enumerate(self.out_names)})
        return res


_RUNNERS = {}


def _get_runner():
    nc = _get_program()
    if id(nc) not in _RUNNERS:
        _RUNNERS[id(nc)] = _Runner(nc)
    return _RUNNERS[id(nc)]


def kernel(input_gate, q_expert_vector, k_expert_vector, Wq, Wk):
    runner = _get_runner()
    xf = _pack_full(input_gate, q_expert_vector, k_expert_vector, Wq, Wk)
    dev_in = runner.put_full([xf])
    outs = runner.run_device(dev_in)
    out = np.asarray(outs[0]).reshape(ROWS)
    return out.reshape(TOK, E).reshape(B, S, E)



# revision 42
# speedup vs baseline: 1.0501x; 1.0501x over previous
"""Trainium2 Bass kernel for nn_AttRouter (MoE attention routing).

Reference computation (per token t, E=16 experts, D=256, A=64):
    Q = Xq @ Wq                  # (E, A)
    K = Xk @ Wk                  # (E, A)
    sim = softmax_k(Q K^T / 8)   # (E, E)
    gate = sim @ g               # (E,)
    out = softmax_E(gate)

Strategy (8 cores, data-parallel over tokens; 1024 tokens = 16384
token-expert rows per core):
  - Load X in natural layout [rows, D], PE-transpose 128x128 blocks,
    project with D on partitions: QT/KT = W^T @ X^T -> [A, rows].
  - Per 128-row tile (8 tokens): simT[(t,k),(t,q)] = KT^T QT via PE with a
    block-diagonal -BIG mask added by a rank-8 matmul of constant one-hot
    factors (kills cross-token terms after exp).
  - exp on ACT (scale 1/8); numerator+denominator in one PE matmul
    against interleaved [g | 1] columns; all 128 tiles' [num|den] pairs
    accumulate in one PSUM bank.
  - Final routing softmax over the 16 experts (partition-block dim) done
    once per core with a handful of PE/DVE/ACT ops on [128, ntiles] data.

Dispatch-overhead design (the dominant cost under the axon tunnel —
device main-loop time is ~64us/core and fully hidden by pipelining;
per-call PJRT/relay overhead is ~0.65ms and sets the floor):
  - ALL per-core inputs (xq|xk interleaved, gate, weights, constants) are
    packed host-side into ONE dram tensor [16608, 512] -> a call carries
    2 operands (x, out) instead of 8 (~60us/operand/call saved).
  - bf16 mask constants are derived on device from the f32 consts.
  - Output operand buffers are created on device once and reused every
    call (kernel overwrites every element; no per-call h2d transfer).
  - partition_id disabled (unused); callable AOT-compiled at init.

Self-contained: hardcodes shapes/sharding; no file reads.
"""

import numpy as np

B, S, E, D, A = 4, 2048, 16, 256, 64
# One 8-core mesh per call. Measured tradeoff: per-call dispatch is
# ~470-510us for 8 devices vs ~370us for 2, but the axon relay keeps only
# ~2 calls in flight, so fewer-core variants (device time 1.5ms/pair)
# cannot amortize their device time across pipelined calls — 8-way keeps
# per-core device time (~350us) hidden under the dispatch floor.
N_CORES = 8                      # cores per call (one mesh)
N_MESHES = 1                     # disjoint meshes among the 8 cores
TOK = B * S                      # 8192 tokens
ROWS = TOK * E                   # 131072 token-expert rows
ROWS_CORE = ROWS // N_CORES      # 16384 rows per core
NTILES = ROWS_CORE // 128        # 128 tiles of 128 rows
BIG = 240.0                      # additive mask; -240/8 = -30 pre-exp

# ---- perf/precision knobs ----
CHUNK_TILES = 8                  # tiles per DMA chunk (8 -> 2 MiB/chunk)
GROUP = 4                        # tiles per projection group (N=512 matmuls)
PROJ_F32R = True                 # float32r projections (full-rate PE)
TRANS_F32R = False               # float32r transposes via SWDGE cast loads
TRANS_BITCAST = False            # ACT-cast chunks to f32r for the PE
                                 # transposes — measured net LOSS on hw
                                 # (490us vs 413us); transposes are not
                                 # the quarter-rate bottleneck assumed
SIM_DT = "f32r"                  # QT/KT + sim matmul dtype: f32|f32r|bf16
                                 # (f32r: full-rate PE, device loop 378
                                 # -> 316us; trans=f32r via SWDGE is a
                                 # net LOSS on hw, keep TRANS_F32R off)
EXP_DT = "f32"                   # exp output (numden lhsT): f32|f32r|bf16
NO_COMPUTE = False               # DMA-only variant (bandwidth floor probe)
NO_TRANS = False                 # skip PE transposes (diagnostic, wrong data)
PS_T_BUFS = 2                    # transpose psum pool bufs
PS_SIM_BUFS = 2                  # sim psum pool bufs (PSUM budget: ps_t 2
                                 # + ps_qk 2 + ps_sim 2 + ps_nd 2 = 8 banks)
REPEAT = 1                       # run the main loop R times (bench slope)


def _make_consts():
    """f32 consts [128, 392]: identity | maskA | maskB | onehot8."""
    c = np.zeros((128, 392), np.float32)
    c[:, 0:128] = np.eye(128, dtype=np.float32)
    r = np.arange(128)
    # maskA[j, row] = 1 where row's token (row//16) == j  (rows 0..7 used)
    c[r // 16, 128 + r] = 1.0
    # maskB[j, row] = -BIG where row//16 != j (rows 0..7 used)
    mb = np.full((8, 128), -BIG, np.float32)
    mb[r // 16, r] = 0.0
    c[0:8, 256:384] = mb
    # onehot8[row, row//16] = 1
    c[r, 384 + r // 16] = 1.0
    return c


def _pack_rows(ntiles):
    """Row offsets of the packed single input tensor [rows, 512] f32:
    xq|xk (nrows x [256|256]), then g, wq, wk, consts as flat rows."""
    nrows = ntiles * 128
    g_rows = ntiles * 128 // 512
    r_g = nrows
    r_wq = r_g + g_rows
    r_wk = r_wq + 32
    r_c = r_wk + 32
    r_end = r_c + 128
    return r_g, r_wq, r_wk, r_c, r_end


def build_program(ntiles=NTILES):
    import concourse.bacc as bacc
    import concourse.tile as tile
    from concourse import mybir

    f32 = mybir.dt.float32
    f32r = mybir.dt.float32r
    bf16 = mybir.dt.bfloat16
    dts = {"f32": f32, "f32r": f32r, "bf16": bf16}
    simdt = dts[SIM_DT]
    expdt = dts[EXP_DT]
    Exp = mybir.ActivationFunctionType.Exp

    nrows = ntiles * 128
    nchunks = ntiles // CHUNK_TILES
    groups_per_chunk = CHUNK_TILES // GROUP

    nc = bacc.Bacc("TRN2", enable_partition_id=False)
    r_g, r_wq, r_wk, r_c, r_end = _pack_rows(ntiles)
    x_d = nc.dram_tensor("x", [r_end, 512], f32, kind="ExternalInput")
    g_d = x_d[r_g:r_wq, :].rearrange("r (a c) -> (r a) c", c=128)
    wq_d = x_d[r_wq:r_wk, :].rearrange("r (a c) -> (r a) c", c=128)
    wk_d = x_d[r_wk:r_c, :].rearrange("r (a c) -> (r a) c", c=128)
    consts_d = x_d[r_c:r_end, 0:392]
    out_d = nc.dram_tensor("out", [ntiles, 128], f32, kind="ExternalOutput")

    def r32(ap):
        return ap.bitcast(f32r)

    with tile.TileContext(nc) as tc:
        with (
            tc.tile_pool(name="singles", bufs=1) as singles,
            tc.tile_pool(name="p_in", bufs=2) as p_in,
            tc.tile_pool(name="p_xt", bufs=2) as p_xt,
            tc.tile_pool(name="p_qk", bufs=2) as p_qk,
            tc.tile_pool(name="p_exp", bufs=3) as p_exp,
            tc.tile_pool(name="p_fin", bufs=1) as p_fin,
            tc.tile_pool(name="ps_t", bufs=PS_T_BUFS, space="PSUM") as ps_t,
            tc.tile_pool(name="ps_qk", bufs=1, space="PSUM") as ps_qk,
            tc.tile_pool(name="ps_sim", bufs=PS_SIM_BUFS, space="PSUM")
            as ps_sim,
            tc.tile_pool(name="ps_nd", bufs=1, space="PSUM") as ps_nd,
        ):
            # ---- one-time setup ----
            consts = singles.tile([128, 392], f32)
            nc.sync.dma_start(consts, consts_d)
            ident = consts[:, 0:128]
            # masks always bf16 (0 / -240 are exact in bf16; 4x faster MM);
            # built on device from the f32 consts to save an input operand
            consts16 = singles.tile([8, 256], bf16)
            nc.vector.tensor_copy(consts16[:, 0:128], consts[0:8, 256:384])
            nc.vector.tensor_copy(consts16[:, 128:256], consts[0:8, 128:256])
            maskB, maskA = consts16[:, 0:128], consts16[:, 128:256]
            oh8 = consts[:, 384:392]
            oh8T = consts[0:8, 128:256]          # == maskA rows: delta(t=j)

            wq_sb = singles.tile([128, 128], f32)
            wk_sb = singles.tile([128, 128], f32)
            nc.sync.dma_start(wq_sb, wq_d)
            nc.sync.dma_start(wk_sb, wk_d)
            if PROJ_F32R:
                wq_r = singles.tile([128, 128], f32r)
                wk_r = singles.tile([128, 128], f32r)
                nc.vector.tensor_copy(wq_r, wq_sb)
                nc.vector.tensor_copy(wk_r, wk_sb)
                wq_sb, wk_sb = wq_r, wk_r
            if TRANS_F32R or TRANS_BITCAST:
                ident_t = singles.tile([128, 128], f32r)
                nc.vector.tensor_copy(ident_t, ident)
            else:
                ident_t = ident

            # gate: load [ntiles, 128] in <=128-row blocks, PE-transpose
            # each into gT [128, ntiles] (PSUM), then interleave with ones
            gT_ps = ps_sim.tile([128, ntiles], f32, tag="sim")
            for blk in range(0, ntiles, 128):
                nb = min(128, ntiles - blk)
                g_sb = singles.tile([nb, 128], f32, tag=f"g{blk}")
                nc.sync.dma_start(g_sb, g_d[blk:blk + nb, :])
                nc.tensor.transpose(gT_ps[:, blk:blk + nb], g_sb,
                                    ident[0:nb, 0:nb])
            gones = singles.tile([128, 2 * ntiles], expdt)
            gones_v = gones.rearrange("p (i two) -> p i two", two=2)
            nc.vector.tensor_copy(gones_v[:, :, 0], gT_ps)
            nc.vector.memset(gones_v[:, :, 1], 1.0)

            num_den_ps = ps_nd.tile([128, 2 * ntiles], f32)

            # ---- main loop ----
            indt = f32r if TRANS_F32R else f32
            tdt = f32r if (TRANS_F32R or TRANS_BITCAST) else f32
            dma_in = nc.gpsimd if TRANS_F32R else nc.sync
            for c in [ci for _ in range(REPEAT) for ci in range(nchunks)]:
                r0 = c * CHUNK_TILES * 128
                r1 = r0 + CHUNK_TILES * 128
                # one DMA for q|k together: 2KB contiguous per (p, n) line
                inx = p_in.tile([128, CHUNK_TILES * 512], indt, tag="inx")
                dma_in.dma_start(
                    inx.rearrange("p (n d) -> p n d", d=512),
                    x_d[r0:r1, :].rearrange("(n p) d -> p n d", p=128))

                if NO_COMPUTE:
                    sink = p_exp.tile([1, 8], f32, tag="sink")
                    nc.vector.tensor_copy(sink[:, 0:8], inx[0:1, 0:8])
                    continue

                if TRANS_BITCAST and not TRANS_F32R:
                    # round the chunk to f32r on the (mostly idle) ACT
                    # engine so the PE transposes run at full rate
                    inxr = p_in.tile([128, CHUNK_TILES * 512], f32r,
                                     tag="inxr")
                    nc.scalar.activation(inxr, inx,
                                         mybir.ActivationFunctionType.Copy,
                                         scale=1.0)
                    inx = inxr

                for gi in range(groups_per_chunk):
                    xtdt = f32r if PROJ_F32R else f32
                    xqT = p_xt.tile([128, GROUP * 256], xtdt, tag="xqT")
                    xkT = p_xt.tile([128, GROUP * 256], xtdt, tag="xkT")
                    for j in range(GROUP):
                        jj = gi * GROUP + j
                        if NO_TRANS:
                            # diagnostic: same copies, wrong (untransposed)
                            # data, no PE transposes
                            nc.vector.tensor_copy(
                                xqT[:, j * 256:(j + 1) * 256],
                                inx[:, jj * 512:jj * 512 + 256])
                            nc.vector.tensor_copy(
                                xkT[:, j * 256:(j + 1) * 256],
                                inx[:, jj * 512 + 256:(jj + 1) * 512])
                            continue
                        pT = ps_t.tile([128, 512], tdt, tag="pT")
                        for h in range(2):
                            src_q = inx[:, jj * 512 + h * 128:
                                        jj * 512 + (h + 1) * 128]
                            src_k = inx[:, jj * 512 + 256 + h * 128:
                                        jj * 512 + 256 + (h + 1) * 128]
                            nc.tensor.transpose(
                                pT[:, h * 128:(h + 1) * 128], src_q, ident_t)
                            nc.tensor.transpose(
                                pT[:, 256 + h * 128:256 + (h + 1) * 128],
                                src_k, ident_t)
                        nc.vector.tensor_copy(
                            xqT[:, j * 256:(j + 1) * 256], pT[:, 0:256])
                        nc.vector.tensor_copy(
                            xkT[:, j * 256:(j + 1) * 256], pT[:, 256:512])

                    # projections: QT/KT [64, GROUP*128], separate banks
                    qt_ps = ps_qk.tile([64, GROUP * 128], f32, tag="qt")
                    kt_ps = ps_qk.tile([64, GROUP * 128], f32, tag="kt")
                    xqT_v = xqT.rearrange("p (j h d) -> p h j d", h=2, d=128)
                    xkT_v = xkT.rearrange("p (j h d) -> p h j d", h=2, d=128)
                    for h in range(2):
                        nc.tensor.matmul(qt_ps,
                                         wq_sb[:, h * 64:(h + 1) * 64],
                                         xqT_v[:, h],
                                         start=(h == 0), stop=(h == 1))
                    for h in range(2):
                        nc.tensor.matmul(kt_ps,
                                         wk_sb[:, h * 64:(h + 1) * 64],
                                         xkT_v[:, h],
                                         start=(h == 0), stop=(h == 1))
                    qt_sb = p_qk.tile([64, GROUP * 128], simdt, tag="qtsb")
                    kt_sb = p_qk.tile([64, GROUP * 128], simdt, tag="ktsb")
                    nc.vector.tensor_copy(qt_sb, qt_ps)
                    nc.vector.tensor_copy(kt_sb, kt_ps)

                    for j in range(GROUP):
                        i = c * CHUNK_TILES + gi * GROUP + j
                        sl = slice(j * 128, (j + 1) * 128)
                        sim_ps = ps_sim.tile([128, 128], f32, tag="sim")
                        nc.tensor.matmul(sim_ps, maskB, maskA,
                                         start=True, stop=False)
                        nc.tensor.matmul(sim_ps, kt_sb[:, sl], qt_sb[:, sl],
                                         start=False, stop=True)
                        exp_t = p_exp.tile([128, 128], expdt, tag="exp")
                        nc.scalar.activation(exp_t, sim_ps, Exp, scale=0.125)
                        nc.tensor.matmul(num_den_ps[:, 2 * i:2 * i + 2],
                                         exp_t, gones[:, 2 * i:2 * i + 2],
                                         start=True, stop=True)

            # ---- final routing softmax over experts ----
            nd_sb = p_fin.tile([128, 2 * ntiles], f32)
            if NO_COMPUTE:
                nc.vector.memset(nd_sb, 1.0)
            else:
                nc.vector.tensor_copy(nd_sb, num_den_ps)
            nd_v = nd_sb.rearrange("p (i two) -> p i two", two=2)
            recd = p_fin.tile([128, ntiles], f32)
            nc.vector.reciprocal(recd, nd_v[:, :, 1])
            eg = p_fin.tile([128, ntiles], f32)
            # egate = exp(num * (1/den)); gate in (0,1) so no max-subtract
            gate = p_fin.tile([128, ntiles], f32)
            nc.vector.tensor_mul(gate, nd_v[:, :, 0], recd)
            nc.scalar.activation(eg, gate, Exp, scale=1.0)

            # per-128-column blocks: expert-sums, reciprocal, transpose
            rsT_sb = p_fin.tile([8, ntiles], f32)
            for blk in range(0, ntiles, 128):
                nb = min(128, ntiles - blk)
                sums_ps = ps_sim.tile([nb, 8], f32, tag="sim")
                nc.tensor.matmul(sums_ps, eg[:, blk:blk + nb], oh8,
                                 start=True, stop=True)
                rs = p_fin.tile([nb, 8], f32, tag=f"rs{blk}")
                nc.vector.reciprocal(rs, sums_ps)
                rsT_ps = ps_sim.tile([8, nb], f32, tag="sim")
                nc.tensor.transpose(rsT_ps, rs, ident[0:nb, 0:nb])
                nc.vector.tensor_copy(rsT_sb[:, blk:blk + nb], rsT_ps)
            bc_ps = ps_sim.tile([128, ntiles], f32, tag="sim")
            nc.tensor.matmul(bc_ps, oh8T, rsT_sb, start=True, stop=True)
            rout = p_fin.tile([128, ntiles], f32)
            nc.vector.tensor_mul(rout, eg, bc_ps)
            for blk in range(0, ntiles, 128):
                nb = min(128, ntiles - blk)
                routT_ps = ps_sim.tile([nb, 128], f32, tag="sim")
                nc.tensor.transpose(routT_ps, rout[:, blk:blk + nb], ident)
                routT_sb = p_fin.tile([nb, 128], f32, tag=f"rt{blk}")
                nc.vector.tensor_copy(routT_sb, routT_ps)
                nc.sync.dma_start(out_d[blk:blk + nb, :], routT_sb)
    return nc


def _pack_full(input_gate, q_expert_vector, k_expert_vector, Wq, Wk,
               ntiles=NTILES):
    """One contiguous packed input [N_CORES * r_end, 512] (single copy)."""
    xq = np.asarray(q_expert_vector, np.float32).reshape(ROWS, D)
    xk = np.asarray(k_expert_vector, np.float32).reshape(ROWS, D)
    g = np.asarray(input_gate, np.float32).reshape(ROWS)
    wq = np.asarray(Wq, np.float32).reshape(2, 128, A).transpose(1, 0, 2) \
        .reshape(128, 128)
    wk = np.asarray(Wk, np.float32).reshape(2, 128, A).transpose(1, 0, 2) \
        .reshape(128, 128)
    consts_pad = np.zeros((128, 512), np.float32)
    consts_pad[:, 0:392] = _make_consts()
    nrows = ntiles * 128
    r_g, r_wq, r_wk, r_c, r_end = _pack_rows(ntiles)
    xf = np.empty((N_CORES, r_end, 512), np.float32)
    if ntiles == NTILES:
        xf[:, 0:nrows, 0:256] = xq.reshape(N_CORES, nrows, D)
        xf[:, 0:nrows, 256:512] = xk.reshape(N_CORES, nrows, D)
        xf[:, r_g:r_wq] = g.reshape(N_CORES, -1, 512)
    else:
        for c in range(N_CORES):
            r0 = c * ROWS_CORE
            xf[c, 0:nrows, 0:256] = xq[r0:r0 + nrows]
            xf[c, 0:nrows, 256:512] = xk[r0:r0 + nrows]
            xf[c, r_g:r_wq] = g[r0:r0 + nrows].reshape(-1, 512)
    xf[:, r_wq:r_wk] = wq.reshape(32, 512)
    xf[:, r_wk:r_c] = wk.reshape(32, 512)
    xf[:, r_c:r_end] = consts_pad
    return xf.reshape(N_CORES * r_end, 512)


def _shard_inputs(input_gate, q_expert_vector, k_expert_vector, Wq, Wk,
                  ntiles=NTILES):
    xf = _pack_full(input_gate, q_expert_vector, k_expert_vector, Wq, Wk,
                    ntiles)
    r_end = _pack_rows(ntiles)[-1]
    xf = xf.reshape(N_CORES, r_end, 512)
    return [{"x": xf[c]} for c in range(N_CORES)]


_CACHE = {}


def _get_program(ntiles=NTILES):
    key = (ntiles, CHUNK_TILES, GROUP, PROJ_F32R, TRANS_F32R, TRANS_BITCAST,
           SIM_DT, EXP_DT, NO_COMPUTE, REPEAT)
    if key not in _CACHE:
        nc = build_program(ntiles)
        nc.finalize()
        _CACHE[key] = nc
    return _CACHE[key]


class _Runner:
    """Cached SPMD PJRT runner (compiles once; repeat calls are cheap).

    Mirrors concourse.bass2jax.run_bass_via_pjrt but keeps the jitted
    callable and mesh alive across calls.
    """

    def __init__(self, nc):
        import jax
        from jax.experimental.shard_map import shard_map
        from jax.sharding import Mesh, PartitionSpec
        from concourse import bass2jax, mybir
        bass2jax.install_neuronx_cc_hook()
        self.jax = jax
        partition_name = (nc.partition_id_tensor.name
                          if nc.partition_id_tensor else None)
        in_names, out_names, out_avals, zero_outs = [], [], [], []
        in_avals = []
        for alloc in nc.m.functions[0].allocations:
            if not isinstance(alloc, mybir.MemoryLocationSet):
                continue
            name = alloc.memorylocations[0].name
            if alloc.kind == "ExternalInput":
                if name != partition_name:
                    in_names.append(name)
                    in_avals.append(jax.core.ShapedArray(
                        tuple(alloc.tensor_shape), mybir.dt.np(alloc.dtype)))
            elif alloc.kind == "ExternalOutput":
                out_names.append(name)
                shape = tuple(alloc.tensor_shape)
                dtype = mybir.dt.np(alloc.dtype)
                out_avals.append(jax.core.ShapedArray(shape, dtype))
                zero_outs.append(np.zeros(shape, dtype))
        self.in_names, self.out_names = list(in_names), list(out_names)
        self.in_avals = in_avals
        self.out_avals, self.zero_outs = out_avals, zero_outs
        n_params, n_outs = len(in_names), len(out_names)
        all_names = list(in_names) + list(out_names)
        if partition_name is not None:
            all_names.append(partition_name)
        all_names = tuple(all_names)

        self._bass2jax = bass2jax
        self._nc = nc
        self._all_names = all_names
        self._partition_name = partition_name
        self._n_params, self._n_outs = n_params, n_outs
        # One mesh per disjoint core pair. Mesh 0 (used by kernel()) is
        # compiled eagerly; the rest lazily on first use (timing loop).
        self._meshes = [None] * N_MESHES
        self._get_mesh(0)

    def _get_mesh(self, m):
        """(compiled_callable, sharding, dev_zeros) for pair m."""
        if self._meshes[m] is not None:
            return self._meshes[m]
        import jax
        from jax.experimental.shard_map import shard_map
        from jax.sharding import Mesh, NamedSharding, PartitionSpec
        bass2jax, nc = self._bass2jax, self._nc
        out_avals, out_names = self.out_avals, self.out_names
        all_names, partition_name = self._all_names, self._partition_name

        def _body(*args):
            operands = list(args)
            if partition_name is not None:
                operands.append(bass2jax.partition_id_tensor())
            outs = bass2jax._bass_exec_p.bind(
                *operands,
                out_avals=tuple(out_avals),
                in_names=all_names,
                out_names=tuple(out_names),
                lowering_input_output_aliases=(),
                sim_require_finite=True,
                sim_require_nnan=True,
                nc=nc,
            )
            return tuple(outs)

        devices = jax.devices()[m * N_CORES:(m + 1) * N_CORES]
        mesh = Mesh(np.asarray(devices), ("core",))
        specs = (PartitionSpec("core"),) * (self._n_params + self._n_outs)
        sharded = jax.jit(
            shard_map(_body, mesh=mesh, in_specs=specs,
                      out_specs=(PartitionSpec("core"),) * self._n_outs,
                      check_rep=False),
            keep_unused=True,
        )
        sh = NamedSharding(mesh, PartitionSpec("core"))
        # Output operand buffers, created on device ONCE and reused every
        # call (no donation). The kernel overwrites every output element,
        # so stale contents are irrelevant; this avoids a per-call
        # host->device zeros transfer over the axon tunnel (~16ms/iter).
        dev_zeros = [
            jax.device_put(
                np.zeros((N_CORES * z.shape[0], *z.shape[1:]), z.dtype), sh)
            for z in self.zero_outs]
        # AOT-compile with bass_effect suppressed: enables jax's C++ pjit
        # fast-path dispatch (the effectful path forces a ~250us/call
        # python fallback + token handling).
        in_sds = [jax.ShapeDtypeStruct(
            (N_CORES * a.shape[0], *a.shape[1:]), a.dtype, sharding=sh)
            for a in self.in_avals]
        out_sds = [jax.ShapeDtypeStruct(
            (N_CORES * z.shape[0], *z.shape[1:]), z.dtype, sharding=sh)
            for z in self.zero_outs]
        compiled = bass2jax.fast_dispatch_compile(
            lambda: sharded.lower(*in_sds, *out_sds).compile())
        self._meshes[m] = (compiled, sh, dev_zeros)
        return self._meshes[m]

    def put_inputs(self, in_maps, mesh_idx=0):
        """Concat per-core inputs and move them to a pair mesh."""
        import jax
        _, sh, _ = self._get_mesh(mesh_idx)
        out = []
        for name in self.in_names:
            arr = np.concatenate([np.asarray(m[name]) for m in in_maps],
                                 axis=0)
            out.append(jax.device_put(arr, sh))
        return out

    def put_full(self, full_arrays, mesh_idx=0):
        """Move pre-packed full input arrays (ordered as in_names) to a
        pair mesh without any host-side copy."""
        import jax
        _, sh, _ = self._get_mesh(mesh_idx)
        return [jax.device_put(a, sh) for a in full_arrays]

    def run_device(self, dev_inputs, mesh_idx=0):
        compiled, _, dev_zeros = self._get_mesh(mesh_idx)
        return compiled(*dev_inputs, *dev_zeros)

    def __call__(self, in_maps):
        outs = self.run_device(self.put_inputs(in_maps))
        res = []
        for c in range(N_CORES):
            res.append({
                name: np.asarray(outs[i]).reshape(
                    N_CORES, *self.out_avals[i].shape)[c]
                for i, name in enumerate(self.out_names)})
        return res


_RUNNERS = {}


def _get_runner():
    nc = _get_program()
    if id(nc) not in _RUNNERS:
        _RUNNERS[id(nc)] = _Runner(nc)
    return _RUNNERS[id(nc)]


def kernel(input_gate, q_expert_vector, k_expert_vector, Wq, Wk):
    runner = _get_runner()
    xf = _pack_full(input_gate, q_expert_vector, k_expert_vector, Wq, Wk)
    dev_in = runner.put_full([xf])
    outs = runner.run_device(dev_in)
    out = np.asarray(outs[0]).reshape(ROWS)
    return out.reshape(TOK, E).reshape(B, S, E)



# revision 50
# speedup vs baseline: 1.2309x; 1.1722x over previous
"""Trainium2 Bass kernel for nn_AttRouter (MoE attention routing).

Reference computation (per token t, E=16 experts, D=256, A=64):
    Q = Xq @ Wq                  # (E, A)
    K = Xk @ Wk                  # (E, A)
    sim = softmax_k(Q K^T / 8)   # (E, E)
    gate = sim @ g               # (E,)
    out = softmax_E(gate)

Strategy (8 cores, data-parallel over tokens; 1024 tokens = 16384
token-expert rows per core):
  - Load X in natural layout [rows, D], PE-transpose 128x128 blocks,
    project with D on partitions: QT/KT = W^T @ X^T -> [A, rows].
  - Per 128-row tile (8 tokens): simT[(t,k),(t,q)] = KT^T QT via PE with a
    block-diagonal -BIG mask added by a rank-8 matmul of constant one-hot
    factors (kills cross-token terms after exp).
  - exp on ACT (scale 1/8); numerator+denominator in one PE matmul
    against interleaved [g | 1] columns; all 128 tiles' [num|den] pairs
    accumulate in one PSUM bank.
  - Final routing softmax over the 16 experts (partition-block dim) done
    once per core with a handful of PE/DVE/ACT ops on [128, ntiles] data.

Dispatch-overhead design (the dominant cost under the axon tunnel —
device main-loop time is ~64us/core and fully hidden by pipelining;
per-call PJRT/relay overhead is ~0.65ms and sets the floor):
  - ALL per-core inputs (xq|xk interleaved, gate, weights, constants) are
    packed host-side into ONE dram tensor [16608, 512] -> a call carries
    2 operands (x, out) instead of 8 (~60us/operand/call saved).
  - bf16 mask constants are derived on device from the f32 consts.
  - Output operand buffers are created on device once and reused every
    call (kernel overwrites every element; no per-call h2d transfer).
  - partition_id disabled (unused); callable AOT-compiled at init.

Self-contained: hardcodes shapes/sharding; no file reads.
"""

import numpy as np

B, S, E, D, A = 4, 2048, 16, 256, 64
# One 8-core mesh per call. Measured tradeoff: per-call dispatch is
# ~470-510us for 8 devices vs ~370us for 2, but the axon relay keeps only
# ~2 calls in flight, so fewer-core variants (device time 1.5ms/pair)
# cannot amortize their device time across pipelined calls — 8-way keeps
# per-core device time (~350us) hidden under the dispatch floor.
N_CORES = 8                      # cores per call (one mesh)
N_MESHES = 1                     # disjoint meshes among the 8 cores
TOK = B * S                      # 8192 tokens
ROWS = TOK * E                   # 131072 token-expert rows
ROWS_CORE = ROWS // N_CORES      # 16384 rows per core
NTILES = ROWS_CORE // 128        # 128 tiles of 128 rows
BIG = 240.0                      # additive mask; -240/8 = -30 pre-exp

# ---- perf/precision knobs ----
CHUNK_TILES = 8                  # tiles per DMA chunk (8 -> 2 MiB/chunk)
GROUP = 4                        # tiles per projection group (N=512 matmuls)
PROJ_F32R = True                 # float32r projections (full-rate PE)
TRANS_F32R = False               # float32r transposes via SWDGE cast loads
TRANS_BITCAST = False            # ACT-cast chunks to f32r for the PE
                                 # transposes — measured net LOSS on hw
                                 # (490us vs 413us); transposes are not
                                 # the quarter-rate bottleneck assumed
SIM_DT = "f32r"                  # QT/KT + sim matmul dtype: f32|f32r|bf16
                                 # (f32r: full-rate PE, device loop 378
                                 # -> 316us; trans=f32r via SWDGE is a
                                 # net LOSS on hw, keep TRANS_F32R off)
EXP_DT = "f32"                   # exp output (numden lhsT): f32|f32r|bf16
NO_COMPUTE = False               # DMA-only variant (bandwidth floor probe)
NO_TRANS = False                 # skip PE transposes (diagnostic, wrong data)
NO_SIM = False                   # stop after projections (ablation probe)
PS_QK_BUFS = 1                   # projection psum pool bufs
PS_T_BUFS = 2                    # transpose psum pool bufs
PS_SIM_BUFS = 2                  # sim psum pool bufs (PSUM budget: ps_t 2
                                 # + ps_qk 2 + ps_sim 2 + ps_nd 2 = 8 banks)
REPEAT = 1                       # run the main loop R times (bench slope)


def _make_consts():
    """f32 consts [128, 392]: identity | maskA | maskB | onehot8."""
    c = np.zeros((128, 392), np.float32)
    c[:, 0:128] = np.eye(128, dtype=np.float32)
    r = np.arange(128)
    # maskA[j, row] = 1 where row's token (row//16) == j  (rows 0..7 used)
    c[r // 16, 128 + r] = 1.0
    # maskB[j, row] = -BIG where row//16 != j (rows 0..7 used)
    mb = np.full((8, 128), -BIG, np.float32)
    mb[r // 16, r] = 0.0
    c[0:8, 256:384] = mb
    # onehot8[row, row//16] = 1
    c[r, 384 + r // 16] = 1.0
    return c


def _pack_rows(ntiles):
    """Row offsets of the packed single input tensor [rows, 512] f32:
    xq|xk (nrows x [256|256]), then g, wq, wk, consts as flat rows."""
    nrows = ntiles * 128
    g_rows = ntiles * 128 // 512
    r_g = nrows
    r_wq = r_g + g_rows
    r_wk = r_wq + 32
    r_c = r_wk + 32
    r_end = r_c + 128
    return r_g, r_wq, r_wk, r_c, r_end


def build_program(ntiles=NTILES):
    import concourse.bacc as bacc
    import concourse.tile as tile
    from concourse import mybir

    f32 = mybir.dt.float32
    f32r = mybir.dt.float32r
    bf16 = mybir.dt.bfloat16
    dts = {"f32": f32, "f32r": f32r, "bf16": bf16}
    simdt = dts[SIM_DT]
    expdt = dts[EXP_DT]
    Exp = mybir.ActivationFunctionType.Exp

    nrows = ntiles * 128
    nchunks = ntiles // CHUNK_TILES
    groups_per_chunk = CHUNK_TILES // GROUP

    nc = bacc.Bacc("TRN2", enable_partition_id=False)
    r_g, r_wq, r_wk, r_c, r_end = _pack_rows(ntiles)
    x_d = nc.dram_tensor("x", [r_end, 512], f32, kind="ExternalInput")
    # xq|xk region is PARTITION-MAJOR (host pre-permuted): dram row
    # p*ntiles + t holds X row t*128 + p. A chunk DMA then reads ONE
    # contiguous 16KB line per partition (vs 8x 2KB with natural order)
    # -- the natural-order layout was DMA-descriptor-bound (~90GB/s).
    xqk_d = x_d[0:nrows, :].rearrange("(p t) d -> p t d", p=128)
    # g region is also partition-major: direct [128, ntiles] view, no
    # PE transpose needed (needs 512 % ntiles == 0)
    g_d = x_d[r_g:r_wq, :].rearrange("r (a c) -> (r a) c", c=min(ntiles, 512))
    wq_d = x_d[r_wq:r_wk, :].rearrange("r (a c) -> (r a) c", c=128)
    wk_d = x_d[r_wk:r_c, :].rearrange("r (a c) -> (r a) c", c=128)
    consts_d = x_d[r_c:r_end, 0:392]
    out_d = nc.dram_tensor("out", [ntiles, 128], f32, kind="ExternalOutput")

    def r32(ap):
        return ap.bitcast(f32r)

    with tile.TileContext(nc) as tc:
        with (
            tc.tile_pool(name="singles", bufs=1) as singles,
            tc.tile_pool(name="p_in", bufs=2) as p_in,
            tc.tile_pool(name="p_xt", bufs=2) as p_xt,
            tc.tile_pool(name="p_qk", bufs=2) as p_qk,
            tc.tile_pool(name="p_exp", bufs=3) as p_exp,
            tc.tile_pool(name="p_fin", bufs=1) as p_fin,
            tc.tile_pool(name="ps_t", bufs=PS_T_BUFS, space="PSUM") as ps_t,
            tc.tile_pool(name="ps_qk", bufs=PS_QK_BUFS, space="PSUM")
            as ps_qk,
            tc.tile_pool(name="ps_sim", bufs=PS_SIM_BUFS, space="PSUM")
            as ps_sim,
            tc.tile_pool(name="ps_nd", bufs=1, space="PSUM") as ps_nd,
        ):
            # ---- one-time setup ----
            consts = singles.tile([128, 392], f32)
            nc.sync.dma_start(consts, consts_d)
            ident = consts[:, 0:128]
            # masks always bf16 (0 / -240 are exact in bf16; 4x faster MM);
            # built on device from the f32 consts to save an input operand
            consts16 = singles.tile([8, 256], bf16)
            nc.vector.tensor_copy(consts16[:, 0:128], consts[0:8, 256:384])
            nc.vector.tensor_copy(consts16[:, 128:256], consts[0:8, 128:256])
            maskB, maskA = consts16[:, 0:128], consts16[:, 128:256]
            oh8 = consts[:, 384:392]
            oh8T = consts[0:8, 128:256]          # == maskA rows: delta(t=j)

            wq_sb = singles.tile([128, 128], f32)
            wk_sb = singles.tile([128, 128], f32)
            nc.sync.dma_start(wq_sb, wq_d)
            nc.sync.dma_start(wk_sb, wk_d)
            if PROJ_F32R:
                wq_r = singles.tile([128, 128], f32r)
                wk_r = singles.tile([128, 128], f32r)
                nc.vector.tensor_copy(wq_r, wq_sb)
                nc.vector.tensor_copy(wk_r, wk_sb)
                wq_sb, wk_sb = wq_r, wk_r
            if TRANS_F32R or TRANS_BITCAST:
                ident_t = singles.tile([128, 128], f32r)
                nc.vector.tensor_copy(ident_t, ident)
            else:
                ident_t = ident

            # gate: stored partition-major in DRAM -> direct [128, ntiles]
            # load, no transpose; interleave with ones
            gT_sb = singles.tile([128, ntiles], f32)
            nc.sync.dma_start(gT_sb, g_d)
            gones = singles.tile([128, 2 * ntiles], expdt)
            gones_v = gones.rearrange("p (i two) -> p i two", two=2)
            nc.vector.tensor_copy(gones_v[:, :, 0], gT_sb)
            nc.vector.memset(gones_v[:, :, 1], 1.0)

            num_den_ps = ps_nd.tile([128, 2 * ntiles], f32)

            # ---- main loop ----
            indt = f32r if TRANS_F32R else f32
            tdt = f32r if (TRANS_F32R or TRANS_BITCAST) else f32
            dma_in = nc.gpsimd if TRANS_F32R else nc.sync
            for c in [ci for _ in range(REPEAT) for ci in range(nchunks)]:
                r0 = c * CHUNK_TILES * 128
                r1 = r0 + CHUNK_TILES * 128
                # one DMA for q|k together: partition-major DRAM layout
                # gives ONE contiguous 16KB line per partition per chunk
                inx = p_in.tile([128, CHUNK_TILES * 512], indt, tag="inx")
                dma_in.dma_start(
                    inx.rearrange("p (n d) -> p n d", d=512),
                    xqk_d[:, c * CHUNK_TILES:(c + 1) * CHUNK_TILES, :])

                if NO_COMPUTE:
                    sink = p_exp.tile([1, 8], f32, tag="sink")
                    nc.vector.tensor_copy(sink[:, 0:8], inx[0:1, 0:8])
                    continue

                if TRANS_BITCAST and not TRANS_F32R:
                    # round the chunk to f32r on the (mostly idle) ACT
                    # engine so the PE transposes run at full rate
                    inxr = p_in.tile([128, CHUNK_TILES * 512], f32r,
                                     tag="inxr")
                    nc.scalar.activation(inxr, inx,
                                         mybir.ActivationFunctionType.Copy,
                                         scale=1.0)
                    inx = inxr

                for gi in range(groups_per_chunk):
                    xtdt = f32r if PROJ_F32R else f32
                    xqT = p_xt.tile([128, GROUP * 256], xtdt, tag="xqT")
                    xkT = p_xt.tile([128, GROUP * 256], xtdt, tag="xkT")
                    for j in range(GROUP):
                        jj = gi * GROUP + j
                        if NO_TRANS:
                            # diagnostic: same copies, wrong (untransposed)
                            # data, no PE transposes
                            nc.vector.tensor_copy(
                                xqT[:, j * 256:(j + 1) * 256],
                                inx[:, jj * 512:jj * 512 + 256])
                            nc.vector.tensor_copy(
                                xkT[:, j * 256:(j + 1) * 256],
                                inx[:, jj * 512 + 256:(jj + 1) * 512])
                            continue
                        pT = ps_t.tile([128, 512], tdt, tag="pT")
                        for h in range(2):
                            src_q = inx[:, jj * 512 + h * 128:
                                        jj * 512 + (h + 1) * 128]
                            src_k = inx[:, jj * 512 + 256 + h * 128:
                                        jj * 512 + 256 + (h + 1) * 128]
                            nc.tensor.transpose(
                                pT[:, h * 128:(h + 1) * 128], src_q, ident_t)
                            nc.tensor.transpose(
                                pT[:, 256 + h * 128:256 + (h + 1) * 128],
                                src_k, ident_t)
                        nc.vector.tensor_copy(
                            xqT[:, j * 256:(j + 1) * 256], pT[:, 0:256])
                        nc.vector.tensor_copy(
                            xkT[:, j * 256:(j + 1) * 256], pT[:, 256:512])

                    # projections: QT/KT [64, GROUP*128], separate banks
                    qt_ps = ps_qk.tile([64, GROUP * 128], f32, tag="qt")
                    kt_ps = ps_qk.tile([64, GROUP * 128], f32, tag="kt")
                    xqT_v = xqT.rearrange("p (j h d) -> p h j d", h=2, d=128)
                    xkT_v = xkT.rearrange("p (j h d) -> p h j d", h=2, d=128)
                    for h in range(2):
                        nc.tensor.matmul(qt_ps,
                                         wq_sb[:, h * 64:(h + 1) * 64],
                                         xqT_v[:, h],
                                         start=(h == 0), stop=(h == 1))
                    for h in range(2):
                        nc.tensor.matmul(kt_ps,
                                         wk_sb[:, h * 64:(h + 1) * 64],
                                         xkT_v[:, h],
                                         start=(h == 0), stop=(h == 1))
                    qt_sb = p_qk.tile([64, GROUP * 128], simdt, tag="qtsb")
                    kt_sb = p_qk.tile([64, GROUP * 128], simdt, tag="ktsb")
                    nc.vector.tensor_copy(qt_sb, qt_ps)
                    nc.vector.tensor_copy(kt_sb, kt_ps)

                    if NO_SIM:
                        sink = p_exp.tile([1, 8], f32, tag="sink")
                        nc.vector.tensor_copy(sink[:, 0:4], qt_sb[0:1, 0:4])
                        nc.vector.tensor_copy(sink[:, 4:8], kt_sb[0:1, 0:4])
                        continue
                    for j in range(GROUP):
                        i = c * CHUNK_TILES + gi * GROUP + j
                        sl = slice(j * 128, (j + 1) * 128)
                        sim_ps = ps_sim.tile([128, 128], f32, tag="sim")
                        nc.tensor.matmul(sim_ps, maskB, maskA,
                                         start=True, stop=False)
                        nc.tensor.matmul(sim_ps, kt_sb[:, sl], qt_sb[:, sl],
                                         start=False, stop=True)
                        exp_t = p_exp.tile([128, 128], expdt, tag="exp")
                        nc.scalar.activation(exp_t, sim_ps, Exp, scale=0.125)
                        nc.tensor.matmul(num_den_ps[:, 2 * i:2 * i + 2],
                                         exp_t, gones[:, 2 * i:2 * i + 2],
                                         start=True, stop=True)

            # ---- final routing softmax over experts ----
            nd_sb = p_fin.tile([128, 2 * ntiles], f32)
            if NO_COMPUTE or NO_SIM:
                nc.vector.memset(nd_sb, 1.0)
            else:
                nc.vector.tensor_copy(nd_sb, num_den_ps)
            nd_v = nd_sb.rearrange("p (i two) -> p i two", two=2)
            recd = p_fin.tile([128, ntiles], f32)
            nc.vector.reciprocal(recd, nd_v[:, :, 1])
            eg = p_fin.tile([128, ntiles], f32)
            # egate = exp(num * (1/den)); gate in (0,1) so no max-subtract
            gate = p_fin.tile([128, ntiles], f32)
            nc.vector.tensor_mul(gate, nd_v[:, :, 0], recd)
            nc.scalar.activation(eg, gate, Exp, scale=1.0)

            # per-128-column blocks: expert-sums, reciprocal, transpose
            rsT_sb = p_fin.tile([8, ntiles], f32)
            for blk in range(0, ntiles, 128):
                nb = min(128, ntiles - blk)
                sums_ps = ps_sim.tile([nb, 8], f32, tag="sim")
                nc.tensor.matmul(sums_ps, eg[:, blk:blk + nb], oh8,
                                 start=True, stop=True)
                rs = p_fin.tile([nb, 8], f32, tag=f"rs{blk}")
                nc.vector.reciprocal(rs, sums_ps)
                rsT_ps = ps_sim.tile([8, nb], f32, tag="sim")
                nc.tensor.transpose(rsT_ps, rs, ident[0:nb, 0:nb])
                nc.vector.tensor_copy(rsT_sb[:, blk:blk + nb], rsT_ps)
            bc_ps = ps_sim.tile([128, ntiles], f32, tag="sim")
            nc.tensor.matmul(bc_ps, oh8T, rsT_sb, start=True, stop=True)
            rout = p_fin.tile([128, ntiles], f32)
            nc.vector.tensor_mul(rout, eg, bc_ps)
            for blk in range(0, ntiles, 128):
                nb = min(128, ntiles - blk)
                routT_ps = ps_sim.tile([nb, 128], f32, tag="sim")
                nc.tensor.transpose(routT_ps, rout[:, blk:blk + nb], ident)
                routT_sb = p_fin.tile([nb, 128], f32, tag=f"rt{blk}")
                nc.vector.tensor_copy(routT_sb, routT_ps)
                nc.sync.dma_start(out_d[blk:blk + nb, :], routT_sb)
    return nc


def _pack_full(input_gate, q_expert_vector, k_expert_vector, Wq, Wk,
               ntiles=NTILES):
    """One contiguous packed input [N_CORES * r_end, 512] (single copy)."""
    xq = np.asarray(q_expert_vector, np.float32).reshape(ROWS, D)
    xk = np.asarray(k_expert_vector, np.float32).reshape(ROWS, D)
    g = np.asarray(input_gate, np.float32).reshape(ROWS)
    wq = np.asarray(Wq, np.float32).reshape(2, 128, A).transpose(1, 0, 2) \
        .reshape(128, 128)
    wk = np.asarray(Wk, np.float32).reshape(2, 128, A).transpose(1, 0, 2) \
        .reshape(128, 128)
    consts_pad = np.zeros((128, 512), np.float32)
    consts_pad[:, 0:392] = _make_consts()
    nrows = ntiles * 128
    r_g, r_wq, r_wk, r_c, r_end = _pack_rows(ntiles)
    # xq|xk and g regions are PARTITION-MAJOR: dram row p*ntiles + t holds
    # X row t*128 + p (matches the kernel's [p, t, d] DMA views)
    xf = np.empty((N_CORES, r_end, 512), np.float32)
    if ntiles == NTILES:
        xf[:, 0:nrows, 0:256] = xq.reshape(N_CORES, ntiles, 128, D) \
            .transpose(0, 2, 1, 3).reshape(N_CORES, nrows, D)
        xf[:, 0:nrows, 256:512] = xk.reshape(N_CORES, ntiles, 128, D) \
            .transpose(0, 2, 1, 3).reshape(N_CORES, nrows, D)
        xf[:, r_g:r_wq] = g.reshape(N_CORES, ntiles, 128) \
            .transpose(0, 2, 1).reshape(N_CORES, -1, 512)
    else:
        for c in range(N_CORES):
            r0 = c * ROWS_CORE
            xf[c, 0:nrows, 0:256] = xq[r0:r0 + nrows] \
                .reshape(ntiles, 128, D).transpose(1, 0, 2) \
                .reshape(nrows, D)
            xf[c, 0:nrows, 256:512] = xk[r0:r0 + nrows] \
                .reshape(ntiles, 128, D).transpose(1, 0, 2) \
                .reshape(nrows, D)
            xf[c, r_g:r_wq] = g[r0:r0 + nrows].reshape(ntiles, 128) \
                .transpose(1, 0).reshape(-1, 512)
    xf[:, r_wq:r_wk] = wq.reshape(32, 512)
    xf[:, r_wk:r_c] = wk.reshape(32, 512)
    xf[:, r_c:r_end] = consts_pad
    return xf.reshape(N_CORES * r_end, 512)


def _shard_inputs(input_gate, q_expert_vector, k_expert_vector, Wq, Wk,
                  ntiles=NTILES):
    xf = _pack_full(input_gate, q_expert_vector, k_expert_vector, Wq, Wk,
                    ntiles)
    r_end = _pack_rows(ntiles)[-1]
    xf = xf.reshape(N_CORES, r_end, 512)
    return [{"x": xf[c]} for c in range(N_CORES)]


_CACHE = {}


def _get_program(ntiles=NTILES):
    key = (ntiles, CHUNK_TILES, GROUP, PROJ_F32R, TRANS_F32R, TRANS_BITCAST,
           SIM_DT, EXP_DT, NO_COMPUTE, NO_SIM, PS_QK_BUFS, PS_T_BUFS,
           PS_SIM_BUFS, REPEAT)
    if key not in _CACHE:
        nc = build_program(ntiles)
        nc.finalize()
        _CACHE[key] = nc
    return _CACHE[key]


class _Runner:
    """Cached SPMD PJRT runner (compiles once; repeat calls are cheap).

    Mirrors concourse.bass2jax.run_bass_via_pjrt but keeps the jitted
    callable and mesh alive across calls.
    """

    def __init__(self, nc):
        import jax
        from jax.experimental.shard_map import shard_map
        from jax.sharding import Mesh, PartitionSpec
        from concourse import bass2jax, mybir
        bass2jax.install_neuronx_cc_hook()
        self.jax = jax
        partition_name = (nc.partition_id_tensor.name
                          if nc.partition_id_tensor else None)
        in_names, out_names, out_avals, zero_outs = [], [], [], []
        in_avals = []
        for alloc in nc.m.functions[0].allocations:
            if not isinstance(alloc, mybir.MemoryLocationSet):
                continue
            name = alloc.memorylocations[0].name
            if alloc.kind == "ExternalInput":
                if name != partition_name:
                    in_names.append(name)
                    in_avals.append(jax.core.ShapedArray(
                        tuple(alloc.tensor_shape), mybir.dt.np(alloc.dtype)))
            elif alloc.kind == "ExternalOutput":
                out_names.append(name)
                shape = tuple(alloc.tensor_shape)
                dtype = mybir.dt.np(alloc.dtype)
                out_avals.append(jax.core.ShapedArray(shape, dtype))
                zero_outs.append(np.zeros(shape, dtype))
        self.in_names, self.out_names = list(in_names), list(out_names)
        self.in_avals = in_avals
        self.out_avals, self.zero_outs = out_avals, zero_outs
        n_params, n_outs = len(in_names), len(out_names)
        all_names = list(in_names) + list(out_names)
        if partition_name is not None:
            all_names.append(partition_name)
        all_names = tuple(all_names)

        self._bass2jax = bass2jax
        self._nc = nc
        self._all_names = all_names
        self._partition_name = partition_name
        self._n_params, self._n_outs = n_params, n_outs
        # One mesh per disjoint core pair. Mesh 0 (used by kernel()) is
        # compiled eagerly; the rest lazily on first use (timing loop).
        self._meshes = [None] * N_MESHES
        self._get_mesh(0)

    def _get_mesh(self, m):
        """(compiled_callable, sharding, dev_zeros) for pair m."""
        if self._meshes[m] is not None:
            return self._meshes[m]
        import jax
        from jax.experimental.shard_map import shard_map
        from jax.sharding import Mesh, NamedSharding, PartitionSpec
        bass2jax, nc = self._bass2jax, self._nc
        out_avals, out_names = self.out_avals, self.out_names
        all_names, partition_name = self._all_names, self._partition_name

        def _body(*args):
            operands = list(args)
            if partition_name is not None:
                operands.append(bass2jax.partition_id_tensor())
            outs = bass2jax._bass_exec_p.bind(
                *operands,
                out_avals=tuple(out_avals),
                in_names=all_names,
                out_names=tuple(out_names),
                lowering_input_output_aliases=(),
                sim_require_finite=True,
                sim_require_nnan=True,
                nc=nc,
            )
            return tuple(outs)

        devices = jax.devices()[m * N_CORES:(m + 1) * N_CORES]
        mesh = Mesh(np.asarray(devices), ("core",))
        specs = (PartitionSpec("core"),) * (self._n_params + self._n_outs)
        sharded = jax.jit(
            shard_map(_body, mesh=mesh, in_specs=specs,
                      out_specs=(PartitionSpec("core"),) * self._n_outs,
                      check_rep=False),
            keep_unused=True,
        )
        sh = NamedSharding(mesh, PartitionSpec("core"))
        # Output operand buffers, created on device ONCE and reused every
        # call (no donation). The kernel overwrites every output element,
        # so stale contents are irrelevant; this avoids a per-call
        # host->device zeros transfer over the axon tunnel (~16ms/iter).
        dev_zeros = [
            jax.device_put(
                np.zeros((N_CORES * z.shape[0], *z.shape[1:]), z.dtype), sh)
            for z in self.zero_outs]
        # AOT-compile with bass_effect suppressed: enables jax's C++ pjit
        # fast-path dispatch (the effectful path forces a ~250us/call
        # python fallback + token handling).
        in_sds = [jax.ShapeDtypeStruct(
            (N_CORES * a.shape[0], *a.shape[1:]), a.dtype, sharding=sh)
            for a in self.in_avals]
        out_sds = [jax.ShapeDtypeStruct(
            (N_CORES * z.shape[0], *z.shape[1:]), z.dtype, sharding=sh)
            for z in self.zero_outs]
        compiled = bass2jax.fast_dispatch_compile(
            lambda: sharded.lower(*in_sds, *out_sds).compile())
        self._meshes[m] = (compiled, sh, dev_zeros)
        return self._meshes[m]

    def put_inputs(self, in_maps, mesh_idx=0):
        """Concat per-core inputs and move them to a pair mesh."""
        import jax
        _, sh, _ = self._get_mesh(mesh_idx)
        out = []
        for name in self.in_names:
            arr = np.concatenate([np.asarray(m[name]) for m in in_maps],
                                 axis=0)
            out.append(jax.device_put(arr, sh))
        return out

    def put_full(self, full_arrays, mesh_idx=0):
        """Move pre-packed full input arrays (ordered as in_names) to a
        pair mesh without any host-side copy."""
        import jax
        _, sh, _ = self._get_mesh(mesh_idx)
        return [jax.device_put(a, sh) for a in full_arrays]

    def run_device(self, dev_inputs, mesh_idx=0):
        compiled, _, dev_zeros = self._get_mesh(mesh_idx)
        return compiled(*dev_inputs, *dev_zeros)

    def __call__(self, in_maps):
        outs = self.run_device(self.put_inputs(in_maps))
        res = []
        for c in range(N_CORES):
            res.append({
                name: np.asarray(outs[i]).reshape(
                    N_CORES, *self.out_avals[i].shape)[c]
                for i, name in enumerate(self.out_names)})
        return res


_RUNNERS = {}


def _get_runner():
    nc = _get_program()
    if id(nc) not in _RUNNERS:
        _RUNNERS[id(nc)] = _Runner(nc)
    return _RUNNERS[id(nc)]


def kernel(input_gate, q_expert_vector, k_expert_vector, Wq, Wk):
    runner = _get_runner()
    xf = _pack_full(input_gate, q_expert_vector, k_expert_vector, Wq, Wk)
    dev_in = runner.put_full([xf])
    outs = runner.run_device(dev_in)
    out = np.asarray(outs[0]).reshape(ROWS)
    return out.reshape(TOK, E).reshape(B, S, E)



# revision 56
# speedup vs baseline: 1.3116x; 1.0656x over previous
"""Trainium2 Bass kernel for nn_AttRouter (MoE attention routing).

Reference computation (per token t, E=16 experts, D=256, A=64):
    Q = Xq @ Wq                  # (E, A)
    K = Xk @ Wk                  # (E, A)
    sim = softmax_k(Q K^T / 8)   # (E, E)
    gate = sim @ g               # (E,)
    out = softmax_E(gate)

Strategy (8 cores, data-parallel over tokens; 1024 tokens = 16384
token-expert rows per core):
  - Load X in natural layout [rows, D], PE-transpose 128x128 blocks,
    project with D on partitions: QT/KT = W^T @ X^T -> [A, rows].
  - Per 128-row tile (8 tokens): simT[(t,k),(t,q)] = KT^T QT via PE with a
    block-diagonal -BIG mask added by a rank-8 matmul of constant one-hot
    factors (kills cross-token terms after exp).
  - exp on ACT (scale 1/8); numerator+denominator in one PE matmul
    against interleaved [g | 1] columns; all 128 tiles' [num|den] pairs
    accumulate in one PSUM bank.
  - Final routing softmax over the 16 experts (partition-block dim) done
    once per core with a handful of PE/DVE/ACT ops on [128, ntiles] data.

Dispatch-overhead design (the dominant cost under the axon tunnel —
device main-loop time is ~64us/core and fully hidden by pipelining;
per-call PJRT/relay overhead is ~0.65ms and sets the floor):
  - ALL per-core inputs (xq|xk interleaved, gate, weights, constants) are
    packed host-side into ONE dram tensor [16608, 512] -> a call carries
    2 operands (x, out) instead of 8 (~60us/operand/call saved).
  - bf16 mask constants are derived on device from the f32 consts.
  - Output operand buffers are created on device once and reused every
    call (kernel overwrites every element; no per-call h2d transfer).
  - partition_id disabled (unused); callable AOT-compiled at init.

Self-contained: hardcodes shapes/sharding; no file reads.
"""

import numpy as np

B, S, E, D, A = 4, 2048, 16, 256, 64
# One 8-core mesh per call. Measured tradeoff: per-call dispatch is
# ~470-510us for 8 devices vs ~370us for 2, but the axon relay keeps only
# ~2 calls in flight, so fewer-core variants (device time 1.5ms/pair)
# cannot amortize their device time across pipelined calls — 8-way keeps
# per-core device time (~350us) hidden under the dispatch floor.
N_CORES = 8                      # cores per call (one mesh)
N_MESHES = 1                     # disjoint meshes among the 8 cores
TOK = B * S                      # 8192 tokens
ROWS = TOK * E                   # 131072 token-expert rows
ROWS_CORE = ROWS // N_CORES      # 16384 rows per core
NTILES = ROWS_CORE // 128        # 128 tiles of 128 rows
BIG = 240.0                      # additive mask; -240/8 = -30 pre-exp

# ---- perf/precision knobs ----
CHUNK_TILES = 8                  # tiles per DMA chunk (8 -> 2 MiB/chunk)
GROUP = 4                        # tiles per projection group (N=512 matmuls)
PROJ_F32R = True                 # float32r projections (full-rate PE)
SIM_DT = "f32r"                  # QT/KT + sim matmul dtype (full-rate PE)
EXP_DT = "f32"                   # exp output (numden lhsT): f32|f32r|bf16
NO_COMPUTE = False               # DMA-only variant (bandwidth floor probe)
NO_SIM = False                   # stop after projections (ablation probe)
PS_QK_BUFS = 1                   # projection psum pool bufs
PS_SIM_BUFS = 2                  # sim psum pool bufs
REPEAT = 1                       # run the main loop R times (bench slope)


def _make_consts():
    """f32 consts [128, 392]: identity | maskA | maskB | onehot8."""
    c = np.zeros((128, 392), np.float32)
    c[:, 0:128] = np.eye(128, dtype=np.float32)
    r = np.arange(128)
    # maskA[j, row] = 1 where row's token (row//16) == j  (rows 0..7 used)
    c[r // 16, 128 + r] = 1.0
    # maskB[j, row] = -BIG where row//16 != j (rows 0..7 used)
    mb = np.full((8, 128), -BIG, np.float32)
    mb[r // 16, r] = 0.0
    c[0:8, 256:384] = mb
    # onehot8[row, row//16] = 1
    c[r, 384 + r // 16] = 1.0
    return c


def _pack_rows(ntiles):
    """Row offsets of the packed single input tensor [rows, 512] f32:
    xqT (nrows/2 rows), xkT (nrows/2), then g, wq, wk, consts."""
    nrows = ntiles * 128
    g_rows = ntiles * 128 // 512
    r_xk = nrows // 2
    r_g = nrows
    r_wq = r_g + g_rows
    r_wk = r_wq + 32
    r_c = r_wk + 32
    r_end = r_c + 128
    return r_xk, r_g, r_wq, r_wk, r_c, r_end


def build_program(ntiles=NTILES):
    import concourse.bacc as bacc
    import concourse.tile as tile
    from concourse import mybir

    f32 = mybir.dt.float32
    f32r = mybir.dt.float32r
    bf16 = mybir.dt.bfloat16
    dts = {"f32": f32, "f32r": f32r, "bf16": bf16}
    simdt = dts[SIM_DT]
    expdt = dts[EXP_DT]
    Exp = mybir.ActivationFunctionType.Exp

    nrows = ntiles * 128
    nchunks = ntiles // CHUNK_TILES
    groups_per_chunk = CHUNK_TILES // GROUP

    nc = bacc.Bacc("TRN2", enable_partition_id=False)
    r_xk, r_g, r_wq, r_wk, r_c, r_end = _pack_rows(ntiles)
    x_d = nc.dram_tensor("x", [r_end, 512], f32, kind="ExternalInput")
    # xqT/xkT regions hold X PRE-TRANSPOSED on the host into the exact
    # SBUF layout the projection consumes: [p, chunk, (j h c)] where
    # element (p, ch, j, h, c) = X[(ch*CT+j)*128 + c, h*128 + p].
    # The kernel then needs NO PE transposes / PSUM round-trip, and each
    # chunk DMA is one contiguous (CT*2048*4)B line per partition.
    xqT_d = x_d[0:r_xk, :].rearrange("(p r) d -> p (r d)", p=128)
    xkT_d = x_d[r_xk:r_g, :].rearrange("(p r) d -> p (r d)", p=128)
    # g region is partition-major: direct [128, ntiles] view, no
    # transpose needed (needs 512 % ntiles == 0)
    g_d = x_d[r_g:r_wq, :].rearrange("r (a c) -> (r a) c", c=min(ntiles, 512))
    wq_d = x_d[r_wq:r_wk, :].rearrange("r (a c) -> (r a) c", c=128)
    wk_d = x_d[r_wk:r_c, :].rearrange("r (a c) -> (r a) c", c=128)
    consts_d = x_d[r_c:r_end, 0:392]
    out_d = nc.dram_tensor("out", [ntiles, 128], f32, kind="ExternalOutput")

    def r32(ap):
        return ap.bitcast(f32r)

    with tile.TileContext(nc) as tc:
        with (
            tc.tile_pool(name="singles", bufs=1) as singles,
            tc.tile_pool(name="p_in", bufs=2) as p_in,
            tc.tile_pool(name="p_qk", bufs=2) as p_qk,
            tc.tile_pool(name="p_exp", bufs=3) as p_exp,
            tc.tile_pool(name="p_fin", bufs=1) as p_fin,
            tc.tile_pool(name="ps_qk", bufs=PS_QK_BUFS, space="PSUM")
            as ps_qk,
            tc.tile_pool(name="ps_sim", bufs=PS_SIM_BUFS, space="PSUM")
            as ps_sim,
            tc.tile_pool(name="ps_nd", bufs=1, space="PSUM") as ps_nd,
        ):
            # ---- one-time setup ----
            consts = singles.tile([128, 392], f32)
            nc.sync.dma_start(consts, consts_d)
            ident = consts[:, 0:128]
            # masks always bf16 (0 / -240 are exact in bf16; 4x faster MM);
            # built on device from the f32 consts to save an input operand
            consts16 = singles.tile([8, 256], bf16)
            nc.vector.tensor_copy(consts16[:, 0:128], consts[0:8, 256:384])
            nc.vector.tensor_copy(consts16[:, 128:256], consts[0:8, 128:256])
            maskB, maskA = consts16[:, 0:128], consts16[:, 128:256]
            oh8 = consts[:, 384:392]
            oh8T = consts[0:8, 128:256]          # == maskA rows: delta(t=j)

            wq_sb = singles.tile([128, 128], f32)
            wk_sb = singles.tile([128, 128], f32)
            nc.sync.dma_start(wq_sb, wq_d)
            nc.sync.dma_start(wk_sb, wk_d)
            if PROJ_F32R:
                wq_r = singles.tile([128, 128], f32r)
                wk_r = singles.tile([128, 128], f32r)
                nc.vector.tensor_copy(wq_r, wq_sb)
                nc.vector.tensor_copy(wk_r, wk_sb)
                wq_sb, wk_sb = wq_r, wk_r

            # gate: stored partition-major in DRAM -> direct [128, ntiles]
            # load, no transpose; interleave with ones
            gT_sb = singles.tile([128, ntiles], f32)
            nc.sync.dma_start(gT_sb, g_d)
            gones = singles.tile([128, 2 * ntiles], expdt)
            gones_v = gones.rearrange("p (i two) -> p i two", two=2)
            nc.vector.tensor_copy(gones_v[:, :, 0], gT_sb)
            nc.vector.memset(gones_v[:, :, 1], 1.0)

            num_den_ps = ps_nd.tile([128, 2 * ntiles], f32)

            # ---- main loop ----
            for c in [ci for _ in range(REPEAT) for ci in range(nchunks)]:
                r0 = c * CHUNK_TILES * 128
                r1 = r0 + CHUNK_TILES * 128
                # chunk DMAs: X arrives pre-transposed and projection-
                # ready; one contiguous 8KB line per partition per tensor
                xdt = f32r if PROJ_F32R else f32
                inq = p_in.tile([128, CHUNK_TILES * 256], xdt, tag="inq")
                ink = p_in.tile([128, CHUNK_TILES * 256], xdt, tag="ink")
                c0 = c * CHUNK_TILES * 256
                c1 = c0 + CHUNK_TILES * 256
                src_q, src_k = xqT_d[:, c0:c1], xkT_d[:, c0:c1]
                if PROJ_F32R:
                    src_q, src_k = r32(src_q), r32(src_k)
                nc.sync.dma_start(inq, src_q)
                nc.sync.dma_start(ink, src_k)

                if NO_COMPUTE:
                    sink = p_exp.tile([1, 8], f32, tag="sink")
                    nc.vector.tensor_copy(sink[:, 0:4], inq[0:1, 0:4])
                    nc.vector.tensor_copy(sink[:, 4:8], ink[0:1, 0:4])
                    continue

                inq_v = inq.rearrange("p (j h d) -> p h j d", h=2, d=128)
                ink_v = ink.rearrange("p (j h d) -> p h j d", h=2, d=128)
                for gi in range(groups_per_chunk):
                    gsl = slice(gi * GROUP, (gi + 1) * GROUP)
                    # projections: QT/KT [64, GROUP*128], separate banks
                    qt_ps = ps_qk.tile([64, GROUP * 128], f32, tag="qt")
                    kt_ps = ps_qk.tile([64, GROUP * 128], f32, tag="kt")
                    for h in range(2):
                        nc.tensor.matmul(qt_ps,
                                         wq_sb[:, h * 64:(h + 1) * 64],
                                         inq_v[:, h, gsl],
                                         start=(h == 0), stop=(h == 1))
                    for h in range(2):
                        nc.tensor.matmul(kt_ps,
                                         wk_sb[:, h * 64:(h + 1) * 64],
                                         ink_v[:, h, gsl],
                                         start=(h == 0), stop=(h == 1))
                    qt_sb = p_qk.tile([64, GROUP * 128], simdt, tag="qtsb")
                    kt_sb = p_qk.tile([64, GROUP * 128], simdt, tag="ktsb")
                    nc.vector.tensor_copy(qt_sb, qt_ps)
                    nc.vector.tensor_copy(kt_sb, kt_ps)

                    if NO_SIM:
                        sink = p_exp.tile([1, 8], f32, tag="sink")
                        nc.vector.tensor_copy(sink[:, 0:4], qt_sb[0:1, 0:4])
                        nc.vector.tensor_copy(sink[:, 4:8], kt_sb[0:1, 0:4])
                        continue
                    for j in range(GROUP):
                        i = c * CHUNK_TILES + gi * GROUP + j
                        sl = slice(j * 128, (j + 1) * 128)
                        sim_ps = ps_sim.tile([128, 128], f32, tag="sim")
                        nc.tensor.matmul(sim_ps, maskB, maskA,
                                         start=True, stop=False)
                        nc.tensor.matmul(sim_ps, kt_sb[:, sl], qt_sb[:, sl],
                                         start=False, stop=True)
                        exp_t = p_exp.tile([128, 128], expdt, tag="exp")
                        nc.scalar.activation(exp_t, sim_ps, Exp, scale=0.125)
                        nc.tensor.matmul(num_den_ps[:, 2 * i:2 * i + 2],
                                         exp_t, gones[:, 2 * i:2 * i + 2],
                                         start=True, stop=True)

            # ---- final routing softmax over experts ----
            nd_sb = p_fin.tile([128, 2 * ntiles], f32)
            if NO_COMPUTE or NO_SIM:
                nc.vector.memset(nd_sb, 1.0)
            else:
                nc.vector.tensor_copy(nd_sb, num_den_ps)
            nd_v = nd_sb.rearrange("p (i two) -> p i two", two=2)
            recd = p_fin.tile([128, ntiles], f32)
            nc.vector.reciprocal(recd, nd_v[:, :, 1])
            eg = p_fin.tile([128, ntiles], f32)
            # egate = exp(num * (1/den)); gate in (0,1) so no max-subtract
            gate = p_fin.tile([128, ntiles], f32)
            nc.vector.tensor_mul(gate, nd_v[:, :, 0], recd)
            nc.scalar.activation(eg, gate, Exp, scale=1.0)

            # per-128-column blocks: expert-sums, reciprocal, transpose
            rsT_sb = p_fin.tile([8, ntiles], f32)
            for blk in range(0, ntiles, 128):
                nb = min(128, ntiles - blk)
                sums_ps = ps_sim.tile([nb, 8], f32, tag="sim")
                nc.tensor.matmul(sums_ps, eg[:, blk:blk + nb], oh8,
                                 start=True, stop=True)
                rs = p_fin.tile([nb, 8], f32, tag=f"rs{blk}")
                nc.vector.reciprocal(rs, sums_ps)
                rsT_ps = ps_sim.tile([8, nb], f32, tag="sim")
                nc.tensor.transpose(rsT_ps, rs, ident[0:nb, 0:nb])
                nc.vector.tensor_copy(rsT_sb[:, blk:blk + nb], rsT_ps)
            bc_ps = ps_sim.tile([128, ntiles], f32, tag="sim")
            nc.tensor.matmul(bc_ps, oh8T, rsT_sb, start=True, stop=True)
            rout = p_fin.tile([128, ntiles], f32)
            nc.vector.tensor_mul(rout, eg, bc_ps)
            for blk in range(0, ntiles, 128):
                nb = min(128, ntiles - blk)
                routT_ps = ps_sim.tile([nb, 128], f32, tag="sim")
                nc.tensor.transpose(routT_ps, rout[:, blk:blk + nb], ident)
                routT_sb = p_fin.tile([nb, 128], f32, tag=f"rt{blk}")
                nc.vector.tensor_copy(routT_sb, routT_ps)
                nc.sync.dma_start(out_d[blk:blk + nb, :], routT_sb)
    return nc


def _pack_full(input_gate, q_expert_vector, k_expert_vector, Wq, Wk,
               ntiles=NTILES):
    """One contiguous packed input [N_CORES * r_end, 512] (single copy)."""
    xq = np.asarray(q_expert_vector, np.float32).reshape(ROWS, D)
    xk = np.asarray(k_expert_vector, np.float32).reshape(ROWS, D)
    g = np.asarray(input_gate, np.float32).reshape(ROWS)
    wq = np.asarray(Wq, np.float32).reshape(2, 128, A).transpose(1, 0, 2) \
        .reshape(128, 128)
    wk = np.asarray(Wk, np.float32).reshape(2, 128, A).transpose(1, 0, 2) \
        .reshape(128, 128)
    consts_pad = np.zeros((128, 512), np.float32)
    consts_pad[:, 0:392] = _make_consts()
    nrows = ntiles * 128
    r_xk, r_g, r_wq, r_wk, r_c, r_end = _pack_rows(ntiles)
    # xqT/xkT regions hold X pre-transposed into the projection-ready
    # SBUF layout: (p, chunk, j, h, c) = X[(chunk*CT+j)*128 + c, h*128+p];
    # g region is partition-major [p, t] (see build_program views)
    nch = ntiles // CHUNK_TILES

    def xt_pack(a):
        # [n*nrows, D] -> [n, r_xk, 512] in (p, ch, j, h, c) order
        n = a.shape[0] // nrows
        return a.reshape(n, nch, CHUNK_TILES, 128, 2, 128) \
            .transpose(0, 5, 1, 2, 4, 3).reshape(n, r_xk, 512)

    xf = np.empty((N_CORES, r_end, 512), np.float32)
    if ntiles == NTILES:
        xf[:, 0:r_xk] = xt_pack(xq)
        xf[:, r_xk:r_g] = xt_pack(xk)
        xf[:, r_g:r_wq] = g.reshape(N_CORES, ntiles, 128) \
            .transpose(0, 2, 1).reshape(N_CORES, -1, 512)
    else:
        for c in range(N_CORES):
            r0 = c * ROWS_CORE
            xf[c, 0:r_xk] = xt_pack(xq[r0:r0 + nrows])[0]
            xf[c, r_xk:r_g] = xt_pack(xk[r0:r0 + nrows])[0]
            xf[c, r_g:r_wq] = g[r0:r0 + nrows].reshape(ntiles, 128) \
                .transpose(1, 0).reshape(-1, 512)
    xf[:, r_wq:r_wk] = wq.reshape(32, 512)
    xf[:, r_wk:r_c] = wk.reshape(32, 512)
    xf[:, r_c:r_end] = consts_pad
    return xf.reshape(N_CORES * r_end, 512)


def _shard_inputs(input_gate, q_expert_vector, k_expert_vector, Wq, Wk,
                  ntiles=NTILES):
    xf = _pack_full(input_gate, q_expert_vector, k_expert_vector, Wq, Wk,
                    ntiles)
    r_end = _pack_rows(ntiles)[-1]
    xf = xf.reshape(N_CORES, r_end, 512)
    return [{"x": xf[c]} for c in range(N_CORES)]


_CACHE = {}


def _get_program(ntiles=NTILES):
    key = (ntiles, CHUNK_TILES, GROUP, PROJ_F32R, SIM_DT, EXP_DT,
           NO_COMPUTE, NO_SIM, PS_QK_BUFS, PS_SIM_BUFS, REPEAT)
    if key not in _CACHE:
        nc = build_program(ntiles)
        nc.finalize()
        _CACHE[key] = nc
    return _CACHE[key]


class _Runner:
    """Cached SPMD PJRT runner (compiles once; repeat calls are cheap).

    Mirrors concourse.bass2jax.run_bass_via_pjrt but keeps the jitted
    callable and mesh alive across calls.
    """

    def __init__(self, nc):
        import jax
        from jax.experimental.shard_map import shard_map
        from jax.sharding import Mesh, PartitionSpec
        from concourse import bass2jax, mybir
        bass2jax.install_neuronx_cc_hook()
        self.jax = jax
        partition_name = (nc.partition_id_tensor.name
                          if nc.partition_id_tensor else None)
        in_names, out_names, out_avals, zero_outs = [], [], [], []
        in_avals = []
        for alloc in nc.m.functions[0].allocations:
            if not isinstance(alloc, mybir.MemoryLocationSet):
                continue
            name = alloc.memorylocations[0].name
            if alloc.kind == "ExternalInput":
                if name != partition_name:
                    in_names.append(name)
                    in_avals.append(jax.core.ShapedArray(
                        tuple(alloc.tensor_shape), mybir.dt.np(alloc.dtype)))
            elif alloc.kind == "ExternalOutput":
                out_names.append(name)
                shape = tuple(alloc.tensor_shape)
                dtype = mybir.dt.np(alloc.dtype)
                out_avals.append(jax.core.ShapedArray(shape, dtype))
                zero_outs.append(np.zeros(shape, dtype))
        self.in_names, self.out_names = list(in_names), list(out_names)
        self.in_avals = in_avals
        self.out_avals, self.zero_outs = out_avals, zero_outs
        n_params, n_outs = len(in_names), len(out_names)
        all_names = list(in_names) + list(out_names)
        if partition_name is not None:
            all_names.append(partition_name)
        all_names = tuple(all_names)

        self._bass2jax = bass2jax
        self._nc = nc
        self._all_names = all_names
        self._partition_name = partition_name
        self._n_params, self._n_outs = n_params, n_outs
        # One mesh per disjoint core pair. Mesh 0 (used by kernel()) is
        # compiled eagerly; the rest lazily on first use (timing loop).
        self._meshes = [None] * N_MESHES
        self._get_mesh(0)

    def _get_mesh(self, m):
        """(compiled_callable, sharding, dev_zeros) for pair m."""
        if self._meshes[m] is not None:
            return self._meshes[m]
        import jax
        from jax.experimental.shard_map import shard_map
        from jax.sharding import Mesh, NamedSharding, PartitionSpec
        bass2jax, nc = self._bass2jax, self._nc
        out_avals, out_names = self.out_avals, self.out_names
        all_names, partition_name = self._all_names, self._partition_name

        def _body(*args):
            operands = list(args)
            if partition_name is not None:
                operands.append(bass2jax.partition_id_tensor())
            outs = bass2jax._bass_exec_p.bind(
                *operands,
                out_avals=tuple(out_avals),
                in_names=all_names,
                out_names=tuple(out_names),
                lowering_input_output_aliases=(),
                sim_require_finite=True,
                sim_require_nnan=True,
                nc=nc,
            )
            return tuple(outs)

        devices = jax.devices()[m * N_CORES:(m + 1) * N_CORES]
        mesh = Mesh(np.asarray(devices), ("core",))
        specs = (PartitionSpec("core"),) * (self._n_params + self._n_outs)
        sharded = jax.jit(
            shard_map(_body, mesh=mesh, in_specs=specs,
                      out_specs=(PartitionSpec("core"),) * self._n_outs,
                      check_rep=False),
            keep_unused=True,
        )
        sh = NamedSharding(mesh, PartitionSpec("core"))
        # Output operand buffers, created on device ONCE and reused every
        # call (no donation). The kernel overwrites every output element,
        # so stale contents are irrelevant; this avoids a per-call
        # host->device zeros transfer over the axon tunnel (~16ms/iter).
        dev_zeros = [
            jax.device_put(
                np.zeros((N_CORES * z.shape[0], *z.shape[1:]), z.dtype), sh)
            for z in self.zero_outs]
        # AOT-compile with bass_effect suppressed: enables jax's C++ pjit
        # fast-path dispatch (the effectful path forces a ~250us/call
        # python fallback + token handling).
        in_sds = [jax.ShapeDtypeStruct(
            (N_CORES * a.shape[0], *a.shape[1:]), a.dtype, sharding=sh)
            for a in self.in_avals]
        out_sds = [jax.ShapeDtypeStruct(
            (N_CORES * z.shape[0], *z.shape[1:]), z.dtype, sharding=sh)
            for z in self.zero_outs]
        compiled = bass2jax.fast_dispatch_compile(
            lambda: sharded.lower(*in_sds, *out_sds).compile())
        self._meshes[m] = (compiled, sh, dev_zeros)
        return self._meshes[m]

    def put_inputs(self, in_maps, mesh_idx=0):
        """Concat per-core inputs and move them to a pair mesh."""
        import jax
        _, sh, _ = self._get_mesh(mesh_idx)
        out = []
        for name in self.in_names:
            arr = np.concatenate([np.asarray(m[name]) for m in in_maps],
                                 axis=0)
            out.append(jax.device_put(arr, sh))
        return out

    def put_full(self, full_arrays, mesh_idx=0):
        """Move pre-packed full input arrays (ordered as in_names) to a
        pair mesh without any host-side copy."""
        import jax
        _, sh, _ = self._get_mesh(mesh_idx)
        return [jax.device_put(a, sh) for a in full_arrays]

    def run_device(self, dev_inputs, mesh_idx=0):
        compiled, _, dev_zeros = self._get_mesh(mesh_idx)
        return compiled(*dev_inputs, *dev_zeros)

    def __call__(self, in_maps):
        outs = self.run_device(self.put_inputs(in_maps))
        res = []
        for c in range(N_CORES):
            res.append({
                name: np.asarray(outs[i]).reshape(
                    N_CORES, *self.out_avals[i].shape)[c]
                for i, name in enumerate(self.out_names)})
        return res


_RUNNERS = {}


def _get_runner():
    nc = _get_program()
    if id(nc) not in _RUNNERS:
        _RUNNERS[id(nc)] = _Runner(nc)
    return _RUNNERS[id(nc)]


def kernel(input_gate, q_expert_vector, k_expert_vector, Wq, Wk):
    runner = _get_runner()
    xf = _pack_full(input_gate, q_expert_vector, k_expert_vector, Wq, Wk)
    dev_in = runner.put_full([xf])
    outs = runner.run_device(dev_in)
    out = np.asarray(outs[0]).reshape(ROWS)
    return out.reshape(TOK, E).reshape(B, S, E)

